# revision 56
# baseline (speedup 1.0000x reference)
"""Trainium2 Bass kernel for nn_FFNet_17600775979626.

Spiking FFN layer: cur = einsum('tbi,oi->tbo', x, W) + b, followed by a
leaky-integrate-and-fire scan over T with subtractive reset (snntorch Leaky,
beta=0.95, threshold=1.0). Returns spk_rec [T, B, NO] (0.0/1.0 floats).

MODE "dr8" (default) — fp16 + fp8-DoubleRow GEMM, output sharding:

  Each core owns a 256-wide slice of the 2048 outputs and all of (T, B).
  GEMM precision scheme (validated vs fp64: cur err std ~1e-5, ~250 spike
  mismatches of 33.5M -> rel err 8.8e-3):
    x = xh + xl, W = Wh + Wl  (fp16 hi + fp32 residual)
    cur ~= xh@Wh                                   # fp16 matmuls, 1 cyc/row
         + 2^-21 * (xh8@Wl8 + xl8@Wh8)             # one fp8 DoubleRow matmul
    where xh8 = fp8(xh), xl8 = fp8(xl*2^16), Wl8 = fp8(Wl*2^21),
    Wh8 = fp8(Wh*2^5). A DoubleRow matmul contracts BOTH correction plane
    pairs in a single pass at 0.5 cyc/row, so a k-chunk costs 256+128 PE
    cycles instead of 3*256 (fp16x2 baseline) — a 2x tensor-engine speedup.
    The dropped xl@Wl term is ~2^-22 relative. See _build_nc_dr8 for the
    dataflow (on-chip xh8 cast, scan structure, engine assignment).

  Scan reformulation (bias folded into per-step feed tensors):
      curb_t = cur_t + b        onem_t = 1 - curb_t     bcur_t = beta*curb_t
      st_t   = w_{t-1} > onem_t
      w_t    = (beta*w_{t-1} + bcur_t) - st_t

MODE "dr" — same GEMM scheme on a 2 (batch) x 4 (output) core grid with two
timesteps per matmul group. REJECTED by the walrus birverifier: its [64, 512]
half-partition scan ops violate the equal-base-partition rule for SBUF ALU
operands. Kept for reference; do not ship.

Walrus codegen on this target accepts at most ONE sync-wait command per
engine instruction, while Tile's wait assigner freely emits several. Two
post-scheduling passes fix that: _slim_waits drops waits already implied
transitively (per-queue FIFO dispatch + semaphore vector clocks), and
_split_waits moves any excess waits onto injected same-queue NoOps.
"""

import os

import numpy as np

T, B, NI, NO = 128, 128, 2048, 2048
NCORES = 8
BGN, OGN = 2, 4  # batch groups x output groups
B_S = B // BGN  # 64 batch rows per core
O_S = NO // OGN  # 512 output neurons per core
KC = NI // 128  # 16 contraction chunks
BETA = 0.95
G = 4  # M-groups (2 steps each) per DMA batch
NG = T * B_S // 128  # 64 M-groups (= T/2)
CSH = 23  # fp8 correction PSUM carries 2^CSH * (xh@Wl + xl@Wh)

MODE = os.environ.get("KERNEL_MODE", "dr8")

_cache = {}

O_S8 = NO // NCORES  # 256 output neurons per core in dr8 mode
TQ8 = 8  # timesteps per DMA batch in dr8 mode
CSH8 = 21  # dr8: correction PSUM carries 2^CSH8 * (xh@Wl + xl@Wh)


def _build_nc_dr8():
    """1x8 output sharding, fp16 + fp8-DoubleRow GEMM, one timestep per
    matmul group.

    Each core owns a 256-wide output slice and all of (T, B). Per step the
    GEMM is 16 fp16 k-chunk matmuls (xh@Wh -> ps_m) plus 16 fp8 DoubleRow
    matmuls contracting both correction plane pairs at 0.5 cyc/row
    (xh8@Wl8 + xl8@Wh8 -> ps_c, scales 2^0*2^21 and 2^16*2^5 = 2^21).
    x ships as fp16 (xh) + one fp8 plane (xl8): xl8 is DMAed straight into
    plane 1 of the per-batch x8c tile, and the xh8 plane is derived on-chip
    (the Act engine does a pure fp16->fp8 copy into plane 0, two steps ahead
    of the PE). That keeps per-core DMA at ~104MB, under the ~340GB/s DMA
    pool at the PE's 327us floor. DMA transfers serialize on a shared engine
    pool in issue order, so batches move as half-batch chunks interleaved in
    consumption order and compute gates on subtile completion sems.

    The LIF scan runs at full width [128, 256] (batch on partitions), so
    every elementwise op has base partition 0 (the walrus birverifier
    rejects SBUF ALU operands with differing base partitions). All six scan
    ops sit on DVE (~2.0us/step < 2.56us PE budget), so any serialization
    the Tile scheduler picks still fits; the PSUM downscale runs on Act.
    """
    from contextlib import ExitStack

    import concourse.bass as bass
    import concourse.mybir as mybir
    import concourse.tile as tile

    f32 = mybir.dt.float32
    f16 = mybir.dt.float16
    f8 = mybir.dt.float8e4
    DR = mybir.MatmulPerfMode.DoubleRow
    TB = T * B
    TQ = TQ8
    NB = T // TQ

    nc = bass.Bass()
    xh = nc.declare_dram_parameter("xh", [NI, TB], f16, isOutput=False)
    xl8 = nc.declare_dram_parameter("xl8", [NI, TB], f8, isOutput=False)
    wh = nc.declare_dram_parameter("wh", [NI, O_S8], f16, isOutput=False)
    w8 = nc.declare_dram_parameter("w8", [2, NI, O_S8], f8, isOutput=False)
    # plane 0: 1 - b (bcast over partitions); plane 1: beta * b
    bt = nc.declare_dram_parameter("bt", [2, 128, O_S8], f32, isOutput=False)
    spk = nc.declare_dram_parameter("spk", [T, B, O_S8], f16, isOutput=True)

    with tile.TileContext(nc) as tc, ExitStack() as ctx:
        singles = ctx.enter_context(tc.tile_pool(name="singles", bufs=1))
        xhp = ctx.enter_context(tc.tile_pool(name="xhp", bufs=2))
        x8cp = ctx.enter_context(tc.tile_pool(name="x8cp", bufs=2))
        stp = ctx.enter_context(tc.tile_pool(name="stp", bufs=2))
        scr = ctx.enter_context(tc.tile_pool(name="scr", bufs=3))
        pmp = ctx.enter_context(tc.tile_pool(name="pmp", bufs=4, space="PSUM"))
        pcp = ctx.enter_context(tc.tile_pool(name="pcp", bufs=4, space="PSUM"))

        xhr = xh[:].rearrange("(k p) tb -> p k tb", p=128)
        xlr = xl8[:].rearrange("(k p) tb -> p k tb", p=128)

        # The x8c batch tile pairs the DoubleRow planes: plane 1 (xl8) is
        # DMAed straight from DRAM; plane 0 (fp8(xh)) is filled per step by
        # an Act-engine fp16->fp8 copy. No on-chip plane shuffling needed.
        # The DMA engines are a shared-bandwidth pool and transfers serialize
        # in issue order, so the preload interleaves operand kinds: the fp16
        # side of the first half-batch lands first (fp16 matmuls start ~9us
        # in), then the fp8 side (DoubleRow groups join at ~12us).
        xh_ts = {0: xhp.tile([128, KC, TQ * 128], f16, name="xh_t")}
        x8c_ts = {0: x8cp.tile([128, 2, KC, TQ * 128], f8, name="x8c_t")}
        wh_sb = singles.tile([128, KC, O_S8], f16)
        w8_sb = singles.tile([128, 2, KC, O_S8], f8)
        bias_sb = singles.tile([128, 2, O_S8], f32)
        h = TQ * 64  # 4-timestep column chunk
        q = TQ * 32  # 2-timestep column chunk
        nc.sync.dma_start(out=xh_ts[0][:, :, :h], in_=xhr[:, :, :h])
        nc.sync.dma_start(out=wh_sb[:], in_=wh[:].rearrange("(k p) o -> p k o", p=128))
        nc.sync.dma_start(out=x8c_ts[0][:, 1, :, :h], in_=xlr[:, :, :h])
        nc.sync.dma_start(
            out=w8_sb[:], in_=w8[:].rearrange("h (k p) o -> p h k o", p=128)
        )
        nc.sync.dma_start(out=bias_sb[:], in_=bt[:].rearrange("h p o -> p h o"))
        nc.sync.dma_start(out=xh_ts[0][:, :, h:], in_=xhr[:, :, h : TQ * 128])
        nc.sync.dma_start(out=x8c_ts[0][:, 1, :, h:], in_=xlr[:, :, h : TQ * 128])

        w_sb = singles.tile([128, O_S8], f32)  # carry: beta*m + b - spk
        nc.vector.memset(w_sb[:], 0.0)

        spk_r = spk[:].rearrange("(gb ti) b o -> gb b ti o", ti=TQ)

        def ensure_batch(gb):
            # Halved, consumption-ordered transfers: matmuls gate on subtile
            # completion sems, so the batch's first 4 steps can start while
            # its second half is still in flight.
            if gb in xh_ts or gb >= NB:
                return
            xh_t = xhp.tile([128, KC, TQ * 128], f16, name="xh_t")
            x8c_t = x8cp.tile([128, 2, KC, TQ * 128], f8, name="x8c_t")
            base = gb * TQ * 128
            for lo, hi in ((0, h), (h, TQ * 128)):
                nc.sync.dma_start(
                    out=xh_t[:, :, lo:hi], in_=xhr[:, :, base + lo : base + hi]
                )
                nc.sync.dma_start(
                    out=x8c_t[:, 1, :, lo:hi], in_=xlr[:, :, base + lo : base + hi]
                )
            xh_ts[gb], x8c_ts[gb] = xh_t, x8c_t

        def emit_cast(t):
            # Fill plane 0 of x8c for step t: a pure fp16->fp8 copy on Act.
            if t >= T:
                return
            gb, ti = divmod(t, TQ)
            cw = slice(ti * 128, (ti + 1) * 128)
            nc.scalar.activation(
                x8c_ts[gb][:, 0, :, cw],
                xh_ts[gb][:, :, cw],
                mybir.ActivationFunctionType.Copy,
            )

        st_ts = {}
        emit_cast(0)
        emit_cast(1)
        for t in range(T):
            gb, ti = divmod(t, TQ)
            if ti == 0:
                st_ts[gb] = stp.tile([128, TQ, O_S8], f16, name="st_t")
            if ti == 0:
                ensure_batch(gb + 1)

            ps_m = pmp.tile([128, O_S8], f32, tag="m")
            cw = slice(ti * 128, (ti + 1) * 128)
            for k in range(KC):
                nc.tensor.matmul(
                    ps_m[:],
                    lhsT=xh_ts[gb][:, k, cw],
                    rhs=wh_sb[:, k, :],
                    start=(k == 0),
                    stop=(k == KC - 1),
                )
            ps_c = pcp.tile([128, O_S8], f32, tag="c")
            for k in range(KC):
                nc.tensor.matmul(
                    ps_c[:],
                    lhsT=x8c_ts[gb][:, :, k, cw],
                    rhs=w8_sb[:, :, k, :],
                    start=(k == 0),
                    stop=(k == KC - 1),
                    perf_mode=DR,
                )
            emit_cast(t + 2)

            # scan: curb = ps_m + 2^-CSH8*ps_c + b (bias folded into the
            # onem/bcur tiles); st = w > 1-curb; w' = (beta*w + beta*curb)-st
            c1 = scr.tile([128, O_S8], f32, tag="c1")
            nc.scalar.activation(
                c1[:], ps_c[:], mybir.ActivationFunctionType.Copy, scale=2.0**-CSH8
            )
            c0 = scr.tile([128, O_S8], f32, tag="c0")
            nc.vector.tensor_tensor(c0[:], c1[:], ps_m[:], mybir.AluOpType.add)
            onem = scr.tile([128, O_S8], f32, tag="onem")
            nc.vector.scalar_tensor_tensor(
                onem[:],
                c0[:],
                -1.0,
                bias_sb[:, 0, :],
                mybir.AluOpType.mult,
                mybir.AluOpType.add,
            )
            bcur = scr.tile([128, O_S8], f32, tag="bcur")
            nc.vector.scalar_tensor_tensor(
                bcur[:],
                c0[:],
                BETA,
                bias_sb[:, 1, :],
                mybir.AluOpType.mult,
                mybir.AluOpType.add,
            )
            stv = st_ts[gb][:, ti, :]
            nc.vector.tensor_tensor(stv, w_sb[:], onem[:], mybir.AluOpType.is_gt)
            if t < T - 1:  # the final carry update is dead code
                p_t = scr.tile([128, O_S8], f32, tag="p")
                nc.vector.scalar_tensor_tensor(
                    p_t[:],
                    w_sb[:],
                    BETA,
                    bcur[:],
                    mybir.AluOpType.mult,
                    mybir.AluOpType.add,
                )
                nc.vector.tensor_tensor(w_sb[:], p_t[:], stv, mybir.AluOpType.subtract)

            # spikes leave in half-batches (quarters at the very end):
            # keeps the out-queue smooth and shortens the final drain tail.
            if ti == TQ // 2 - 1:
                nc.sync.dma_start(
                    out=spk_r[gb, :, : TQ // 2], in_=st_ts[gb][:, : TQ // 2, :]
                )
            elif gb == NB - 1 and ti == 5:
                nc.sync.dma_start(out=spk_r[gb, :, 4:6], in_=st_ts[gb][:, 4:6, :])
            elif ti == TQ - 1:
                lo = 6 if gb == NB - 1 else TQ // 2
                nc.sync.dma_start(
                    out=spk_r[gb, :, lo:], in_=st_ts[gb][:, lo:, :]
                )

    _slim_waits(nc)
    _split_waits(nc)
    return nc


def _prepare_in_maps_dr8(x, W, b):
    import ml_dtypes

    f8 = ml_dtypes.float8_e4m3
    x = np.ascontiguousarray(x, dtype=np.float32)
    W = np.ascontiguousarray(W, dtype=np.float32)
    b = np.ascontiguousarray(b, dtype=np.float32)

    x2 = x.reshape(T * B, NI)
    xT = np.ascontiguousarray(x2.T)
    xh = xT.astype(np.float16)
    xl8 = ((xT - xh.astype(np.float32)) * 2.0**16).astype(f8)

    in_maps = []
    for c in range(NCORES):
        Wc = W[c * O_S8 : (c + 1) * O_S8, :]
        WT = np.ascontiguousarray(Wc.T)  # [NI, O_S8]
        Wh = WT.astype(np.float16)
        Wl = WT - Wh.astype(np.float32)
        w8 = np.empty((2, NI, O_S8), f8)
        w8[0] = (Wl * 2.0**21).astype(f8)  # pairs with fp8(xh) (plane 0)
        w8[1] = (Wh.astype(np.float32) * 2.0**5).astype(f8)  # pairs with xl8
        bc = b[c * O_S8 : (c + 1) * O_S8]
        bt = np.empty((2, 128, O_S8), np.float32)
        bt[0] = 1.0 - bc
        bt[1] = BETA * bc
        in_maps.append({"xh": xh, "xl8": xl8, "wh": Wh, "w8": w8, "bt": bt})
    return in_maps


def _build_nc_dr():
    from contextlib import ExitStack

    import concourse.bass as bass
    import concourse.mybir as mybir
    import concourse.tile as tile

    f32 = mybir.dt.float32
    f16 = mybir.dt.float16
    f8 = mybir.dt.float8e4
    DR = mybir.MatmulPerfMode.DoubleRow
    TB = T * B_S

    nc = bass.Bass()
    xh = nc.declare_dram_parameter("xh", [NI, TB], f16, isOutput=False)
    x8 = nc.declare_dram_parameter("x8", [2, NI, TB], f8, isOutput=False)
    wh = nc.declare_dram_parameter("wh", [NI, O_S], f16, isOutput=False)
    w8 = nc.declare_dram_parameter("w8", [2, NI, O_S], f8, isOutput=False)
    # plane 0: 1 - b (bcast over partitions); plane 1: beta * b
    bt = nc.declare_dram_parameter("bt", [2, 128, O_S], f32, isOutput=False)
    spk = nc.declare_dram_parameter("spk", [T, B_S, O_S], f16, isOutput=True)

    with tile.TileContext(nc) as tc, ExitStack() as ctx:
        singles = ctx.enter_context(tc.tile_pool(name="singles", bufs=1))
        xhp = ctx.enter_context(tc.tile_pool(name="xhp", bufs=2))
        x8p = ctx.enter_context(tc.tile_pool(name="x8p", bufs=2))
        stp = ctx.enter_context(tc.tile_pool(name="stp", bufs=2))
        scr = ctx.enter_context(tc.tile_pool(name="scr", bufs=2))
        pmp = ctx.enter_context(tc.tile_pool(name="pmp", bufs=3, space="PSUM"))
        pcp = ctx.enter_context(tc.tile_pool(name="pcp", bufs=4, space="PSUM"))

        xhr = xh[:].rearrange("(k p) tb -> p k tb", p=128)
        x8r = x8[:].rearrange("h (k p) tb -> p h k tb", p=128)

        # DMA issue order sets arrival order on the queue: the fp16 operands
        # (xh batch 0, Wh) first so pass-1 matmuls start ~11us in, then the
        # fp8 operands for the DoubleRow groups.
        xh_t0 = xhp.tile([128, KC, G * 128], f16)
        nc.sync.dma_start(out=xh_t0[:], in_=xhr[:, :, : G * 128])
        wh_sb = singles.tile([128, KC, O_S], f16)
        nc.sync.dma_start(out=wh_sb[:], in_=wh[:].rearrange("(k p) o -> p k o", p=128))
        x8_t0 = x8p.tile([128, 2, KC, G * 128], f8)
        nc.sync.dma_start(out=x8_t0[:], in_=x8r[:, :, :, : G * 128])
        w8_sb = singles.tile([128, 2, KC, O_S], f8)
        nc.sync.dma_start(
            out=w8_sb[:], in_=w8[:].rearrange("h (k p) o -> p h k o", p=128)
        )
        bias_sb = singles.tile([128, 2, O_S], f32)
        nc.sync.dma_start(out=bias_sb[:], in_=bt[:].rearrange("h p o -> p h o"))

        w_sb = singles.tile([64, O_S], f32)  # carry: beta*m - spk, per (b, o)
        nc.vector.memset(w_sb[:], 0.0)

        spk_r = spk[:].rearrange("(gb gi s) b o -> gb (s b) gi o", gi=G, s=2)

        def emit_f(xh_t, gi):
            ps_m = pmp.tile([128, O_S], f32, tag="m")
            cw = slice(gi * 128, (gi + 1) * 128)
            for k in range(KC):
                nc.tensor.matmul(
                    ps_m[:],
                    lhsT=xh_t[:, k, cw],
                    rhs=wh_sb[:, k, :],
                    start=(k == 0),
                    stop=(k == KC - 1),
                )
            return ps_m

        def emit_d(x8_t, gi):
            ps_c = pcp.tile([128, O_S], f32, tag="c")
            cw = slice(gi * 128, (gi + 1) * 128)
            for k in range(KC):
                nc.tensor.matmul(
                    ps_c[:],
                    lhsT=x8_t[:, :, k, cw],
                    rhs=w8_sb[:, :, k, :],
                    start=(k == 0),
                    stop=(k == KC - 1),
                    perf_mode=DR,
                )
            return ps_c

        def emit_feeds(ps_m, ps_c):
            # curb = ps_m + 2^-CSH*ps_c + b, then the bias-folded scan
            # tensors; stt ops cannot take two PSUM sources, so the otherwise
            # idle Act engine downscales the correction PSUM.
            c1 = scr.tile([128, O_S], f32, tag="c1", bufs=3)
            nc.scalar.activation(
                c1[:], ps_c[:], mybir.ActivationFunctionType.Copy, scale=2.0**-CSH
            )
            c0 = scr.tile([128, O_S], f32, tag="c0", bufs=3)
            nc.vector.tensor_tensor(c0[:], c1[:], ps_m[:], mybir.AluOpType.add)
            onem = scr.tile([128, O_S], f32, tag="onem", bufs=3)
            nc.gpsimd.scalar_tensor_tensor(
                onem[:],
                c0[:],
                -1.0,
                bias_sb[:, 0, :],
                mybir.AluOpType.mult,
                mybir.AluOpType.add,
            )
            bcur = scr.tile([128, O_S], f32, tag="bcur", bufs=3)
            nc.gpsimd.scalar_tensor_tensor(
                bcur[:],
                c0[:],
                BETA,
                bias_sb[:, 1, :],
                mybir.AluOpType.mult,
                mybir.AluOpType.add,
            )
            return onem, bcur

        def emit_state(onem, bcur, st_t, gi):
            for s in range(2):
                ph = slice(s * 64, (s + 1) * 64)
                stv = st_t[ph, gi, :]
                nc.vector.tensor_tensor(stv, w_sb[:], onem[ph, :], mybir.AluOpType.is_gt)
                p_t = scr.tile([64, O_S], f32, tag="p", bufs=4)
                nc.gpsimd.scalar_tensor_tensor(
                    p_t[:],
                    w_sb[:],
                    BETA,
                    bcur[ph, :],
                    mybir.AluOpType.mult,
                    mybir.AluOpType.add,
                )
                nc.vector.tensor_tensor(w_sb[:], p_t[:], stv, mybir.AluOpType.subtract)

        # The scan is software-pipelined one M-group deep: group g's feed ops
        # (PSUM combine + bias folds, no serial state dependency) are emitted
        # BEFORE group g-1's state-update ops so the engine FIFOs never force
        # the feeds behind the w-chain. The true critical cycle is then just
        # st/p -> w per sub-step (~3.4us), under the PE's 5.1us per group.
        PIPE = 2  # scan pipeline depth in M-groups
        pending = []  # deferred (onem, bcur, st_t, gi, gb) awaiting state ops
        xh_t = x8_t = st_t = None
        ps_ms = {}
        for g in range(NG):
            gb, gi = divmod(g, G)
            if gi == 0:
                if gb == 0:
                    xh_t, x8_t = xh_t0, x8_t0
                else:
                    xh_t = xhp.tile([128, KC, G * 128], f16)
                    nc.sync.dma_start(
                        out=xh_t[:], in_=xhr[:, :, gb * G * 128 : (gb + 1) * G * 128]
                    )
                    x8_t = x8p.tile([128, 2, KC, G * 128], f8)
                    nc.sync.dma_start(
                        out=x8_t[:],
                        in_=x8r[:, :, :, gb * G * 128 : (gb + 1) * G * 128],
                    )
                st_t = stp.tile([128, G, O_S], f16)
            if gb == 0:
                # Batch 0: run fp16 groups up to 3 ahead of the DoubleRow
                # groups so the PE (in-order) isn't idled by the fp8 operand
                # preload, which queues behind the fp16 one on the DMA queue.
                # The 3-ahead fp16 group is emitted after this group's DR
                # matmuls: its PSUM slot frees only once this group's PSUM
                # combine has run, which itself needs the DR result.
                if g == 0:
                    for ahead in range(3):
                        ps_ms[ahead] = emit_f(xh_t, ahead)
                ps_m = ps_ms.pop(g)
                ps_c = emit_d(x8_t, gi)
                if g + 3 < G:
                    ps_ms[g + 3] = emit_f(xh_t, g + 3)
            else:
                ps_m = emit_f(xh_t, gi)
                ps_c = emit_d(x8_t, gi)
            onem, bcur = emit_feeds(ps_m, ps_c)
            pending.append((onem, bcur, st_t, gi, gb))
            if len(pending) > PIPE:
                po, pb, pst, pgi, pgb = pending.pop(0)
                emit_state(po, pb, pst, pgi)
                if pgi == G - 1:  # finished writing batch pgb's st tile
                    nc.sync.dma_start(out=spk_r[pgb], in_=pst[:])
        for po, pb, pst, pgi, pgb in pending:
            emit_state(po, pb, pst, pgi)
            if pgi == G - 1:
                nc.sync.dma_start(out=spk_r[pgb], in_=pst[:])

    _slim_waits(nc)
    _split_waits(nc)
    return nc


def _build_nc_fp16x2():
    """Previous-generation kernel: pure output sharding, fp16x2 3-pass GEMM.

    Kept for A/B timing. O_S8 = 256 outputs per core, x replicated.
    """
    from contextlib import ExitStack

    import concourse.bass as bass
    import concourse.mybir as mybir
    import concourse.tile as tile

    f32 = mybir.dt.float32
    dt_mm = mybir.dt.float16
    O_S8 = NO // NCORES
    KC8 = NI // 128

    nc = bass.Bass()
    n_planes = 2
    xT = nc.declare_dram_parameter("xT", [n_planes, NI, T * B], dt_mm, isOutput=False)
    WTs = nc.declare_dram_parameter("WTs", [n_planes, NI, O_S8], dt_mm, isOutput=False)
    ob = nc.declare_dram_parameter(
        "ob", [1, 128 + n_planes * O_S8], dt_mm, isOutput=False
    )
    spk = nc.declare_dram_parameter("spk", [T, B, O_S8], f32, isOutput=True)

    TQ = 4
    with tile.TileContext(nc) as tc, ExitStack() as ctx:
        singles = ctx.enter_context(tc.tile_pool(name="singles", bufs=1))
        xpool = ctx.enter_context(tc.tile_pool(name="xp", bufs=2))
        spool = ctx.enter_context(tc.tile_pool(name="sp", bufs=3))
        sbpool = ctx.enter_context(tc.tile_pool(name="sb", bufs=2))
        psum = ctx.enter_context(tc.tile_pool(name="ps", bufs=6, space="PSUM"))

        xTr = xT[:].rearrange("h (k p) tb -> p h k tb", p=128)
        xt0 = xpool.tile([128, n_planes, KC8, TQ * B], dt_mm)
        nc.sync.dma_start(out=xt0[:], in_=xTr[:, :, :, : TQ * B])
        wt_sb = singles.tile([128, n_planes, KC8, O_S8], dt_mm)
        WTr = WTs[:].rearrange("h (k p) o -> p h k o", p=128)
        for h in range(n_planes):
            nc.sync.dma_start(out=wt_sb[:, h], in_=WTr[:, h])
        ob_sb = singles.tile([1, 128 + n_planes * O_S8], dt_mm)
        nc.sync.dma_start(out=ob_sb[:], in_=ob[:])

        m_sb = singles.tile([128, O_S8], f32)
        w_sb = singles.tile([128, O_S8], f32)
        bias_full = singles.tile([128, O_S8], f32)
        ps_b = psum.tile([128, O_S8], f32, tag="c")
        for h in range(n_planes):
            nc.tensor.matmul(
                ps_b[:],
                lhsT=ob_sb[:, :128],
                rhs=ob_sb[:, 128 + h * O_S8 : 128 + (h + 1) * O_S8],
                start=(h == 0),
                stop=(h == n_planes - 1),
            )
        nc.vector.tensor_copy(bias_full[:], ps_b[:])
        nc.vector.tensor_copy(w_sb[:], bias_full[:])

        spk_r = spk[:].rearrange("(tq tt) b o -> tq b tt o", tt=TQ)

        for tq in range(T // TQ):
            if tq == 0:
                xt = xt0
            else:
                xt = xpool.tile([128, n_planes, KC8, TQ * B], dt_mm)
                nc.sync.dma_start(
                    out=xt[:], in_=xTr[:, :, :, tq * TQ * B : (tq + 1) * TQ * B]
                )
            st = spool.tile([128, TQ, O_S8], f32)

            for tt in range(TQ):
                ps = psum.tile([128, O_S8], f32, tag="c")
                passes = ((0, 0), (0, 1), (1, 0))
                mms = [(k, hx, hw) for k in range(KC8) for hx, hw in passes]
                for i, (k, hx, hw) in enumerate(mms):
                    nc.tensor.matmul(
                        ps[:],
                        lhsT=xt[:, hx, k, tt * B : (tt + 1) * B],
                        rhs=wt_sb[:, hw, k, :],
                        start=(i == 0),
                        stop=(i == len(mms) - 1),
                    )
                nc.vector.tensor_tensor(m_sb[:], w_sb[:], ps[:], mybir.AluOpType.add)
                nc.vector.tensor_scalar(
                    st[:, tt, :], m_sb[:], 1.0, None, mybir.AluOpType.is_gt
                )
                sb = sbpool.tile([128, O_S8], f32)
                nc.vector.tensor_tensor(
                    sb[:], st[:, tt, :], bias_full[:], mybir.AluOpType.subtract
                )
                nc.vector.scalar_tensor_tensor(
                    w_sb[:],
                    m_sb[:],
                    BETA,
                    sb[:],
                    mybir.AluOpType.mult,
                    mybir.AluOpType.subtract,
                )
            nc.sync.dma_start(out=spk_r[tq], in_=st[:])

    _slim_waits(nc)
    _split_waits(nc)
    return nc


def _build_nc(mode):
    if mode == "dr8":
        return _build_nc_dr8()
    if mode == "dr":
        return _build_nc_dr()
    return _build_nc_fp16x2()


def _slim_waits(nc):
    """Drop sync waits already implied by earlier ones (transitive closure).

    Each engine queue dispatches in FIFO order, so a wait satisfied on an
    earlier instruction of the same queue covers later instructions. A wait
    on sem s >= v also imports everything the incrementing instruction's
    queue had itself waited for when it raised s to v (semaphore vector
    clocks with snapshots at each increment).
    """
    FRAMEWORK_OPS = ("InstEventSemaphore", "InstDrain")
    engine_clock = {}  # engine -> {sem_id: value known reached}
    totals = {}  # sem_id -> running total of increments
    snapshots = {}  # sem_id -> [(value, clock dict)] in increasing value order
    poisoned = set()  # sems touched by non-monotonic updates (barriers)

    def join(dst, src):
        for s, v in src.items():
            if s in poisoned:
                continue
            if dst.get(s, -1) < v:
                dst[s] = v

    for blk in nc.m.functions[0].blocks:
        for inst in blk.instructions:
            si = getattr(inst, "sync_info", None)
            if si is None:
                continue
            is_framework = type(inst).__name__ in FRAMEWORK_OPS
            clock = engine_clock.setdefault(inst.engine, {})
            if si.on_wait:
                kept = []
                for w in si.on_wait:
                    if (
                        w.sync_type != "semaphore"
                        or w.wait_mode != "sem-ge-imm"
                        or w.id in poisoned
                    ):
                        kept.append(w)
                        continue
                    covered = clock.get(w.id, -1) >= w.wait_value
                    for val, snap in snapshots.get(w.id, ()):
                        if val <= w.wait_value:
                            join(clock, snap)
                        else:
                            break
                    if clock.get(w.id, -1) < w.wait_value:
                        clock[w.id] = w.wait_value
                    if is_framework or not covered:
                        kept.append(w)
                si.on_wait = kept
            if si.on_update:
                for u in si.on_update:
                    if u.sync_type != "semaphore":
                        continue
                    if u.update_mode not in ("sem-inc", "sem-add-imm"):
                        # barrier-style sem: stop reasoning about it entirely
                        poisoned.add(u.id)
                        totals.pop(u.id, None)
                        snapshots.pop(u.id, None)
                        for c in engine_clock.values():
                            c.pop(u.id, None)
                        continue
                    if u.id in poisoned:
                        continue
                    tot = totals.get(u.id, 0) + (u.update_value or 1)
                    totals[u.id] = tot
                    snap = dict(clock)
                    snap[u.id] = tot
                    snapshots.setdefault(u.id, []).append((tot, snap))


def _split_waits(nc, limit=1):
    """Move excess sync waits onto injected same-queue NoOps.

    Walrus codegen accepts at most `limit` sync-wait commands per engine
    instruction on this target. Engine queues dispatch in order, so a
    preceding NoOp carrying the wait is equivalent.
    """
    import concourse.mybir as mybir

    n_nops = 0
    for blk in nc.m.functions[0].blocks:
        out = []
        changed = False
        for inst in blk.instructions:
            si = getattr(inst, "sync_info", None)
            if type(inst).__name__ == "InstEventSemaphore":
                out.append(inst)
                continue
            if si is not None and si.on_wait and len(si.on_wait) > limit:
                waits = list(si.on_wait)
                for w in waits[:-limit]:
                    nop = mybir.InstNoOp(name=f"wnop-{n_nops}", ins=[], outs=[])
                    n_nops += 1
                    nop.engine = inst.engine
                    nop.sync_info = mybir.SyncInfo(on_wait=[w], on_update=[])
                    nop.bass_nofuse = True
                    out.append(nop)
                    changed = True
                si.on_wait = waits[-limit:]
            out.append(inst)
        if changed:
            try:
                blk.instructions = out
            except Exception:
                blk.instructions.clear()
                blk.instructions.extend(out)


def _split16(a):
    hi = a.astype(np.float16)
    lo = (a - hi.astype(np.float32)).astype(np.float16)
    return hi, lo


def _prepare_in_maps_dr(x, W, b):
    import ml_dtypes

    f8 = ml_dtypes.float8_e4m3
    x = np.ascontiguousarray(x, dtype=np.float32)
    W = np.ascontiguousarray(W, dtype=np.float32)
    b = np.ascontiguousarray(b, dtype=np.float32)

    # per batch group: xh [NI, T*B_S] fp16, x8 [2, NI, T*B_S] fp8
    xh_bg, x8_bg = [], []
    for bg in range(BGN):
        xc = x[:, bg * B_S : (bg + 1) * B_S, :].reshape(T * B_S, NI)
        xT = np.ascontiguousarray(xc.T)
        xh = xT.astype(np.float16)
        xl = xT - xh.astype(np.float32)
        x8 = np.empty((2, NI, T * B_S), f8)
        x8[0] = (xh.astype(np.float32) * 2.0**4).astype(f8)
        x8[1] = (xl * 2.0**16).astype(f8)
        xh_bg.append(xh)
        x8_bg.append(x8)

    # per output group: wh [NI, O_S] fp16, w8 [2, NI, O_S] fp8, bias tiles
    wh_og, w8_og, bt_og = [], [], []
    for og in range(OGN):
        Wc = W[og * O_S : (og + 1) * O_S, :]
        WT = np.ascontiguousarray(Wc.T)  # [NI, O_S]
        Wh = WT.astype(np.float16)
        Wl = WT - Wh.astype(np.float32)
        w8 = np.empty((2, NI, O_S), f8)
        w8[0] = (Wl * 2.0**19).astype(f8)  # pairs with xh8 (plane 0)
        w8[1] = (Wh.astype(np.float32) * 2.0**7).astype(f8)  # pairs with xl8
        bc = b[og * O_S : (og + 1) * O_S]
        bt = np.empty((2, 128, O_S), np.float32)
        bt[0] = 1.0 - bc
        bt[1] = BETA * bc
        wh_og.append(Wh)
        w8_og.append(w8)
        bt_og.append(bt)

    in_maps = []
    for c in range(NCORES):
        bg, og = divmod(c, OGN)
        in_maps.append(
            {
                "xh": xh_bg[bg],
                "x8": x8_bg[bg],
                "wh": wh_og[og],
                "w8": w8_og[og],
                "bt": bt_og[og],
            }
        )
    return in_maps


def _prepare_in_maps_fp16x2(x, W, b):
    O_S8 = NO // NCORES
    x = np.ascontiguousarray(x, dtype=np.float32)
    W = np.ascontiguousarray(W, dtype=np.float32)
    b = np.ascontiguousarray(b, dtype=np.float32)
    x2 = x.reshape(T * B, NI)
    xh, xl = _split16(x2)
    xT = np.stack([np.ascontiguousarray(xh.T), np.ascontiguousarray(xl.T)])
    Wh, Wl = _split16(W)
    WTs_full = np.stack([np.ascontiguousarray(Wh.T), np.ascontiguousarray(Wl.T)])
    bh, bl = _split16(b)
    b_planes = [bh, bl]
    n_planes = 2
    in_maps = []
    for c in range(NCORES):
        ob = np.empty((1, 128 + n_planes * O_S8), np.float16)
        ob[0, :128] = 1.0
        for h in range(n_planes):
            ob[0, 128 + h * O_S8 : 128 + (h + 1) * O_S8] = b_planes[h][
                c * O_S8 : (c + 1) * O_S8
            ]
        in_maps.append(
            {
                "xT": xT,
                "WTs": np.ascontiguousarray(WTs_full[:, :, c * O_S8 : (c + 1) * O_S8]),
                "ob": ob,
            }
        )
    return in_maps


def run(x, W, b, trace=False):
    """Run the kernel; returns (out [T,B,NO] fp32, BassKernelResults)."""
    from concourse.bass_utils import run_bass_kernel_spmd

    if MODE not in _cache:
        _cache[MODE] = _build_nc(MODE)
    nc = _cache[MODE]
    if MODE == "dr8":
        in_maps = _prepare_in_maps_dr8(x, W, b)
    elif MODE == "dr":
        in_maps = _prepare_in_maps_dr(x, W, b)
    else:
        in_maps = _prepare_in_maps_fp16x2(x, W, b)
    res = run_bass_kernel_spmd(nc, in_maps, list(range(NCORES)), trace=trace)
    if MODE == "dr8":
        out = np.empty((T, B, NO), np.float32)
        for c in range(NCORES):
            out[:, :, c * O_S8 : (c + 1) * O_S8] = res.results[c]["spk"].astype(
                np.float32
            )
    elif MODE == "dr":
        out = np.empty((T, B, NO), np.float32)
        for c in range(NCORES):
            bg, og = divmod(c, OGN)
            out[:, bg * B_S : (bg + 1) * B_S, og * O_S : (og + 1) * O_S] = res.results[
                c
            ]["spk"].astype(np.float32)
    else:
        out = np.concatenate([res.results[c]["spk"] for c in range(NCORES)], axis=2)
    return out, res


def kernel(x, W, b):
    out, _ = run(x, W, b, trace=False)
    return out


# revision 57
# speedup vs baseline: 1.0078x; 1.0078x over previous
"""Trainium2 Bass kernel for nn_FFNet_17600775979626.

Spiking FFN layer: cur = einsum('tbi,oi->tbo', x, W) + b, followed by a
leaky-integrate-and-fire scan over T with subtractive reset (snntorch Leaky,
beta=0.95, threshold=1.0). Returns spk_rec [T, B, NO] (0.0/1.0 floats).

MODE "dr8" (default) — fp16 + fp8-DoubleRow GEMM, output sharding:

  Each core owns a 256-wide slice of the 2048 outputs and all of (T, B).
  GEMM precision scheme (validated vs fp64: cur err std ~1e-5, ~250 spike
  mismatches of 33.5M -> rel err 8.8e-3):
    x = xh + xl, W = Wh + Wl  (fp16 hi + fp32 residual)
    cur ~= xh@Wh                                   # fp16 matmuls, 1 cyc/row
         + 2^-21 * (xh8@Wl8 + xl8@Wh8)             # one fp8 DoubleRow matmul
    where xh8 = fp8(xh), xl8 = fp8(xl*2^16), Wl8 = fp8(Wl*2^21),
    Wh8 = fp8(Wh*2^5). A DoubleRow matmul contracts BOTH correction plane
    pairs in a single pass at 0.5 cyc/row, so a k-chunk costs 256+128 PE
    cycles instead of 3*256 (fp16x2 baseline) — a 2x tensor-engine speedup.
    The dropped xl@Wl term is ~2^-22 relative. See _build_nc_dr8 for the
    dataflow (on-chip xh8 cast, scan structure, engine assignment).

  Scan reformulation (bias folded into per-step feed tensors):
      curb_t = cur_t + b        onem_t = 1 - curb_t     bcur_t = beta*curb_t
      st_t   = w_{t-1} > onem_t
      w_t    = (beta*w_{t-1} + bcur_t) - st_t

MODE "dr" — same GEMM scheme on a 2 (batch) x 4 (output) core grid with two
timesteps per matmul group. REJECTED by the walrus birverifier: its [64, 512]
half-partition scan ops violate the equal-base-partition rule for SBUF ALU
operands. Kept for reference; do not ship.

Walrus codegen on this target accepts at most ONE sync-wait command per
engine instruction, while Tile's wait assigner freely emits several. Two
post-scheduling passes fix that: _slim_waits drops waits already implied
transitively (per-queue FIFO dispatch + semaphore vector clocks), and
_split_waits moves any excess waits onto injected same-queue NoOps.
"""

import os

import numpy as np

T, B, NI, NO = 128, 128, 2048, 2048
NCORES = 8
BGN, OGN = 2, 4  # batch groups x output groups
B_S = B // BGN  # 64 batch rows per core
O_S = NO // OGN  # 512 output neurons per core
KC = NI // 128  # 16 contraction chunks
BETA = 0.95
G = 4  # M-groups (2 steps each) per DMA batch
NG = T * B_S // 128  # 64 M-groups (= T/2)
CSH = 23  # fp8 correction PSUM carries 2^CSH * (xh@Wl + xl@Wh)

MODE = os.environ.get("KERNEL_MODE", "dr8")

_cache = {}

O_S8 = NO // NCORES  # 256 output neurons per core in dr8 mode
TQ8 = 8  # timesteps per DMA batch in dr8 mode
CSH8 = 21  # dr8: correction PSUM carries 2^CSH8 * (xh@Wl + xl@Wh)


def _build_nc_dr8():
    """1x8 output sharding, fp16 + fp8-DoubleRow GEMM, one timestep per
    matmul group.

    Each core owns a 256-wide output slice and all of (T, B). Per step the
    GEMM is 16 fp16 k-chunk matmuls (xh@Wh -> ps_m) plus 16 fp8 DoubleRow
    matmuls contracting both correction plane pairs at 0.5 cyc/row
    (xh8@Wl8 + xl8@Wh8 -> ps_c, scales 2^0*2^21 and 2^16*2^5 = 2^21).
    x ships as fp16 (xh) + one fp8 plane (xl8): xl8 is DMAed straight into
    plane 1 of the per-batch x8c tile, and the xh8 plane is derived on-chip
    (the Act engine does a pure fp16->fp8 copy into plane 0, two steps ahead
    of the PE). That keeps per-core DMA at ~104MB, under the ~340GB/s DMA
    pool at the PE's 327us floor. DMA transfers serialize on a shared engine
    pool in issue order, so batches move as half-batch chunks interleaved in
    consumption order and compute gates on subtile completion sems.

    The LIF scan runs at full width [128, 256] (batch on partitions), so
    every elementwise op has base partition 0 (the walrus birverifier
    rejects SBUF ALU operands with differing base partitions). All six scan
    ops sit on DVE (~2.0us/step < 2.56us PE budget), so any serialization
    the Tile scheduler picks still fits; the PSUM downscale runs on Act.
    """
    from contextlib import ExitStack

    import concourse.bass as bass
    import concourse.mybir as mybir
    import concourse.tile as tile

    f32 = mybir.dt.float32
    f16 = mybir.dt.float16
    f8 = mybir.dt.float8e4
    DR = mybir.MatmulPerfMode.DoubleRow
    TB = T * B
    TQ = TQ8
    NB = T // TQ

    nc = bass.Bass()
    xh = nc.declare_dram_parameter("xh", [NI, TB], f16, isOutput=False)
    xl8 = nc.declare_dram_parameter("xl8", [NI, TB], f8, isOutput=False)
    wh = nc.declare_dram_parameter("wh", [NI, O_S8], f16, isOutput=False)
    w8l = nc.declare_dram_parameter("w8l", [NI, O_S8], f8, isOutput=False)
    # plane 0: 1 - b (bcast over partitions); plane 1: beta * b
    bt = nc.declare_dram_parameter("bt", [2, 128, O_S8], f32, isOutput=False)
    spk = nc.declare_dram_parameter("spk", [T, B, O_S8], f16, isOutput=True)

    with tile.TileContext(nc) as tc, ExitStack() as ctx:
        singles = ctx.enter_context(tc.tile_pool(name="singles", bufs=1))
        xhp = ctx.enter_context(tc.tile_pool(name="xhp", bufs=2))
        x8cp = ctx.enter_context(tc.tile_pool(name="x8cp", bufs=2))
        stp = ctx.enter_context(tc.tile_pool(name="stp", bufs=2))
        scr = ctx.enter_context(tc.tile_pool(name="scr", bufs=3))
        pmp = ctx.enter_context(tc.tile_pool(name="pmp", bufs=4, space="PSUM"))
        pcp = ctx.enter_context(tc.tile_pool(name="pcp", bufs=4, space="PSUM"))

        xhr = xh[:].rearrange("(k p) tb -> p k tb", p=128)
        xlr = xl8[:].rearrange("(k p) tb -> p k tb", p=128)

        # The x8c batch tile pairs the DoubleRow planes: plane 1 (xl8) is
        # DMAed straight from DRAM; plane 0 (fp8(xh)) is filled per step by
        # an Act-engine fp16->fp8 copy. No on-chip plane shuffling needed.
        # The DMA engines are a shared-bandwidth pool and transfers serialize
        # in issue order, so the preload interleaves operand kinds: the fp16
        # side of the first half-batch lands first (fp16 matmuls start ~9us
        # in), then the fp8 side (DoubleRow groups join at ~12us).
        xh_ts = {0: xhp.tile([128, KC, TQ * 128], f16, name="xh_t")}
        x8c_ts = {0: x8cp.tile([128, 2, KC, TQ * 128], f8, name="x8c_t")}
        wh_sb = singles.tile([128, KC, O_S8], f16)
        w8_sb = singles.tile([128, 2, KC, O_S8], f8)
        bias_sb = singles.tile([128, 2, O_S8], f32)
        h = TQ * 64  # 4-timestep column chunk
        q = TQ * 32  # 2-timestep column chunk
        nc.sync.dma_start(out=xh_ts[0][:, :, :h], in_=xhr[:, :, :h])
        nc.sync.dma_start(out=wh_sb[:], in_=wh[:].rearrange("(k p) o -> p k o", p=128))
        nc.sync.dma_start(out=x8c_ts[0][:, 1, :, :h], in_=xlr[:, :, :h])
        nc.sync.dma_start(
            out=w8_sb[:, 0], in_=w8l[:].rearrange("(k p) o -> p k o", p=128)
        )
        nc.sync.dma_start(out=bias_sb[:], in_=bt[:].rearrange("h p o -> p h o"))
        nc.sync.dma_start(out=xh_ts[0][:, :, h:], in_=xhr[:, :, h : TQ * 128])
        nc.sync.dma_start(out=x8c_ts[0][:, 1, :, h:], in_=xlr[:, :, h : TQ * 128])

        w_sb = singles.tile([128, O_S8], f32)  # carry: beta*m + b - spk
        nc.vector.memset(w_sb[:], 0.0)

        spk_r = spk[:].rearrange("(gb ti) b o -> gb b ti o", ti=TQ)

        def ensure_batch(gb):
            # Halved, consumption-ordered transfers: matmuls gate on subtile
            # completion sems, so the batch's first 4 steps can start while
            # its second half is still in flight.
            if gb in xh_ts or gb >= NB:
                return
            xh_t = xhp.tile([128, KC, TQ * 128], f16, name="xh_t")
            x8c_t = x8cp.tile([128, 2, KC, TQ * 128], f8, name="x8c_t")
            base = gb * TQ * 128
            for lo, hi in ((0, h), (h, TQ * 128)):
                nc.sync.dma_start(
                    out=xh_t[:, :, lo:hi], in_=xhr[:, :, base + lo : base + hi]
                )
                nc.sync.dma_start(
                    out=x8c_t[:, 1, :, lo:hi], in_=xlr[:, :, base + lo : base + hi]
                )
            xh_ts[gb], x8c_ts[gb] = xh_t, x8c_t

        def emit_cast(t):
            # Fill plane 0 of x8c for step t: a pure fp16->fp8 copy on Act.
            if t >= T:
                return
            gb, ti = divmod(t, TQ)
            cw = slice(ti * 128, (ti + 1) * 128)
            nc.scalar.activation(
                x8c_ts[gb][:, 0, :, cw],
                xh_ts[gb][:, :, cw],
                mybir.ActivationFunctionType.Copy,
            )

        st_ts = {}
        emit_cast(0)
        emit_cast(1)
        # Wh8 plane of the DoubleRow weights is derived on-chip from the
        # fp16 Wh (same Act cast as the x plane, scale 2^5), saving 0.5MB of
        # preload traffic. Two halves so early DR k-chunks start sooner.
        for lo, hi in ((0, KC // 2), (KC // 2, KC)):
            nc.scalar.activation(
                w8_sb[:, 1, lo:hi, :],
                wh_sb[:, lo:hi, :],
                mybir.ActivationFunctionType.Copy,
                scale=2.0**5,
            )
        for t in range(T):
            gb, ti = divmod(t, TQ)
            if ti == 0:
                st_ts[gb] = stp.tile([128, TQ, O_S8], f16, name="st_t")
            if ti == 0:
                ensure_batch(gb + 1)

            ps_m = pmp.tile([128, O_S8], f32, tag="m")
            cw = slice(ti * 128, (ti + 1) * 128)
            for k in range(KC):
                nc.tensor.matmul(
                    ps_m[:],
                    lhsT=xh_ts[gb][:, k, cw],
                    rhs=wh_sb[:, k, :],
                    start=(k == 0),
                    stop=(k == KC - 1),
                )
            ps_c = pcp.tile([128, O_S8], f32, tag="c")
            for k in range(KC):
                nc.tensor.matmul(
                    ps_c[:],
                    lhsT=x8c_ts[gb][:, :, k, cw],
                    rhs=w8_sb[:, :, k, :],
                    start=(k == 0),
                    stop=(k == KC - 1),
                    perf_mode=DR,
                )
            emit_cast(t + 2)

            # scan: curb = ps_m + 2^-CSH8*ps_c + b (bias folded into the
            # onem/bcur tiles); st = w > 1-curb; w' = (beta*w + beta*curb)-st
            c1 = scr.tile([128, O_S8], f32, tag="c1")
            nc.scalar.activation(
                c1[:], ps_c[:], mybir.ActivationFunctionType.Copy, scale=2.0**-CSH8
            )
            c0 = scr.tile([128, O_S8], f32, tag="c0")
            nc.vector.tensor_tensor(c0[:], c1[:], ps_m[:], mybir.AluOpType.add)
            onem = scr.tile([128, O_S8], f32, tag="onem")
            nc.vector.scalar_tensor_tensor(
                onem[:],
                c0[:],
                -1.0,
                bias_sb[:, 0, :],
                mybir.AluOpType.mult,
                mybir.AluOpType.add,
            )
            bcur = scr.tile([128, O_S8], f32, tag="bcur")
            nc.vector.scalar_tensor_tensor(
                bcur[:],
                c0[:],
                BETA,
                bias_sb[:, 1, :],
                mybir.AluOpType.mult,
                mybir.AluOpType.add,
            )
            stv = st_ts[gb][:, ti, :]
            nc.vector.tensor_tensor(stv, w_sb[:], onem[:], mybir.AluOpType.is_gt)
            if t < T - 1:  # the final carry update is dead code
                p_t = scr.tile([128, O_S8], f32, tag="p")
                nc.vector.scalar_tensor_tensor(
                    p_t[:],
                    w_sb[:],
                    BETA,
                    bcur[:],
                    mybir.AluOpType.mult,
                    mybir.AluOpType.add,
                )
                nc.vector.tensor_tensor(w_sb[:], p_t[:], stv, mybir.AluOpType.subtract)

            # spikes leave in half-batches (quarters at the very end):
            # keeps the out-queue smooth and shortens the final drain tail.
            if ti == TQ // 2 - 1:
                nc.sync.dma_start(
                    out=spk_r[gb, :, : TQ // 2], in_=st_ts[gb][:, : TQ // 2, :]
                )
            elif gb == NB - 1 and ti == 5:
                nc.sync.dma_start(out=spk_r[gb, :, 4:6], in_=st_ts[gb][:, 4:6, :])
            elif ti == TQ - 1:
                lo = 6 if gb == NB - 1 else TQ // 2
                nc.sync.dma_start(
                    out=spk_r[gb, :, lo:], in_=st_ts[gb][:, lo:, :]
                )

    _slim_waits(nc)
    _split_waits(nc)
    return nc


def _prepare_in_maps_dr8(x, W, b):
    import ml_dtypes

    f8 = ml_dtypes.float8_e4m3
    x = np.ascontiguousarray(x, dtype=np.float32)
    W = np.ascontiguousarray(W, dtype=np.float32)
    b = np.ascontiguousarray(b, dtype=np.float32)

    x2 = x.reshape(T * B, NI)
    xT = np.ascontiguousarray(x2.T)
    xh = xT.astype(np.float16)
    xl8 = ((xT - xh.astype(np.float32)) * 2.0**16).astype(f8)

    in_maps = []
    for c in range(NCORES):
        Wc = W[c * O_S8 : (c + 1) * O_S8, :]
        WT = np.ascontiguousarray(Wc.T)  # [NI, O_S8]
        Wh = WT.astype(np.float16)
        Wl = WT - Wh.astype(np.float32)
        w8l = (Wl * 2.0**21).astype(f8)  # pairs with fp8(xh) (plane 0)
        bc = b[c * O_S8 : (c + 1) * O_S8]
        bt = np.empty((2, 128, O_S8), np.float32)
        bt[0] = 1.0 - bc
        bt[1] = BETA * bc
        in_maps.append({"xh": xh, "xl8": xl8, "wh": Wh, "w8l": w8l, "bt": bt})
    return in_maps


def _build_nc_dr():
    from contextlib import ExitStack

    import concourse.bass as bass
    import concourse.mybir as mybir
    import concourse.tile as tile

    f32 = mybir.dt.float32
    f16 = mybir.dt.float16
    f8 = mybir.dt.float8e4
    DR = mybir.MatmulPerfMode.DoubleRow
    TB = T * B_S

    nc = bass.Bass()
    xh = nc.declare_dram_parameter("xh", [NI, TB], f16, isOutput=False)
    x8 = nc.declare_dram_parameter("x8", [2, NI, TB], f8, isOutput=False)
    wh = nc.declare_dram_parameter("wh", [NI, O_S], f16, isOutput=False)
    w8 = nc.declare_dram_parameter("w8", [2, NI, O_S], f8, isOutput=False)
    # plane 0: 1 - b (bcast over partitions); plane 1: beta * b
    bt = nc.declare_dram_parameter("bt", [2, 128, O_S], f32, isOutput=False)
    spk = nc.declare_dram_parameter("spk", [T, B_S, O_S], f16, isOutput=True)

    with tile.TileContext(nc) as tc, ExitStack() as ctx:
        singles = ctx.enter_context(tc.tile_pool(name="singles", bufs=1))
        xhp = ctx.enter_context(tc.tile_pool(name="xhp", bufs=2))
        x8p = ctx.enter_context(tc.tile_pool(name="x8p", bufs=2))
        stp = ctx.enter_context(tc.tile_pool(name="stp", bufs=2))
        scr = ctx.enter_context(tc.tile_pool(name="scr", bufs=2))
        pmp = ctx.enter_context(tc.tile_pool(name="pmp", bufs=3, space="PSUM"))
        pcp = ctx.enter_context(tc.tile_pool(name="pcp", bufs=4, space="PSUM"))

        xhr = xh[:].rearrange("(k p) tb -> p k tb", p=128)
        x8r = x8[:].rearrange("h (k p) tb -> p h k tb", p=128)

        # DMA issue order sets arrival order on the queue: the fp16 operands
        # (xh batch 0, Wh) first so pass-1 matmuls start ~11us in, then the
        # fp8 operands for the DoubleRow groups.
        xh_t0 = xhp.tile([128, KC, G * 128], f16)
        nc.sync.dma_start(out=xh_t0[:], in_=xhr[:, :, : G * 128])
        wh_sb = singles.tile([128, KC, O_S], f16)
        nc.sync.dma_start(out=wh_sb[:], in_=wh[:].rearrange("(k p) o -> p k o", p=128))
        x8_t0 = x8p.tile([128, 2, KC, G * 128], f8)
        nc.sync.dma_start(out=x8_t0[:], in_=x8r[:, :, :, : G * 128])
        w8_sb = singles.tile([128, 2, KC, O_S], f8)
        nc.sync.dma_start(
            out=w8_sb[:, 0], in_=w8l[:].rearrange("(k p) o -> p k o", p=128)
        )
        bias_sb = singles.tile([128, 2, O_S], f32)
        nc.sync.dma_start(out=bias_sb[:], in_=bt[:].rearrange("h p o -> p h o"))

        w_sb = singles.tile([64, O_S], f32)  # carry: beta*m - spk, per (b, o)
        nc.vector.memset(w_sb[:], 0.0)

        spk_r = spk[:].rearrange("(gb gi s) b o -> gb (s b) gi o", gi=G, s=2)

        def emit_f(xh_t, gi):
            ps_m = pmp.tile([128, O_S], f32, tag="m")
            cw = slice(gi * 128, (gi + 1) * 128)
            for k in range(KC):
                nc.tensor.matmul(
                    ps_m[:],
                    lhsT=xh_t[:, k, cw],
                    rhs=wh_sb[:, k, :],
                    start=(k == 0),
                    stop=(k == KC - 1),
                )
            return ps_m

        def emit_d(x8_t, gi):
            ps_c = pcp.tile([128, O_S], f32, tag="c")
            cw = slice(gi * 128, (gi + 1) * 128)
            for k in range(KC):
                nc.tensor.matmul(
                    ps_c[:],
                    lhsT=x8_t[:, :, k, cw],
                    rhs=w8_sb[:, :, k, :],
                    start=(k == 0),
                    stop=(k == KC - 1),
                    perf_mode=DR,
                )
            return ps_c

        def emit_feeds(ps_m, ps_c):
            # curb = ps_m + 2^-CSH*ps_c + b, then the bias-folded scan
            # tensors; stt ops cannot take two PSUM sources, so the otherwise
            # idle Act engine downscales the correction PSUM.
            c1 = scr.tile([128, O_S], f32, tag="c1", bufs=3)
            nc.scalar.activation(
                c1[:], ps_c[:], mybir.ActivationFunctionType.Copy, scale=2.0**-CSH
            )
            c0 = scr.tile([128, O_S], f32, tag="c0", bufs=3)
            nc.vector.tensor_tensor(c0[:], c1[:], ps_m[:], mybir.AluOpType.add)
            onem = scr.tile([128, O_S], f32, tag="onem", bufs=3)
            nc.gpsimd.scalar_tensor_tensor(
                onem[:],
                c0[:],
                -1.0,
                bias_sb[:, 0, :],
                mybir.AluOpType.mult,
                mybir.AluOpType.add,
            )
            bcur = scr.tile([128, O_S], f32, tag="bcur", bufs=3)
            nc.gpsimd.scalar_tensor_tensor(
                bcur[:],
                c0[:],
                BETA,
                bias_sb[:, 1, :],
                mybir.AluOpType.mult,
                mybir.AluOpType.add,
            )
            return onem, bcur

        def emit_state(onem, bcur, st_t, gi):
            for s in range(2):
                ph = slice(s * 64, (s + 1) * 64)
                stv = st_t[ph, gi, :]
                nc.vector.tensor_tensor(stv, w_sb[:], onem[ph, :], mybir.AluOpType.is_gt)
                p_t = scr.tile([64, O_S], f32, tag="p", bufs=4)
                nc.gpsimd.scalar_tensor_tensor(
                    p_t[:],
                    w_sb[:],
                    BETA,
                    bcur[ph, :],
                    mybir.AluOpType.mult,
                    mybir.AluOpType.add,
                )
                nc.vector.tensor_tensor(w_sb[:], p_t[:], stv, mybir.AluOpType.subtract)

        # The scan is software-pipelined one M-group deep: group g's feed ops
        # (PSUM combine + bias folds, no serial state dependency) are emitted
        # BEFORE group g-1's state-update ops so the engine FIFOs never force
        # the feeds behind the w-chain. The true critical cycle is then just
        # st/p -> w per sub-step (~3.4us), under the PE's 5.1us per group.
        PIPE = 2  # scan pipeline depth in M-groups
        pending = []  # deferred (onem, bcur, st_t, gi, gb) awaiting state ops
        xh_t = x8_t = st_t = None
        ps_ms = {}
        for g in range(NG):
            gb, gi = divmod(g, G)
            if gi == 0:
                if gb == 0:
                    xh_t, x8_t = xh_t0, x8_t0
                else:
                    xh_t = xhp.tile([128, KC, G * 128], f16)
                    nc.sync.dma_start(
                        out=xh_t[:], in_=xhr[:, :, gb * G * 128 : (gb + 1) * G * 128]
                    )
                    x8_t = x8p.tile([128, 2, KC, G * 128], f8)
                    nc.sync.dma_start(
                        out=x8_t[:],
                        in_=x8r[:, :, :, gb * G * 128 : (gb + 1) * G * 128],
                    )
                st_t = stp.tile([128, G, O_S], f16)
            if gb == 0:
                # Batch 0: run fp16 groups up to 3 ahead of the DoubleRow
                # groups so the PE (in-order) isn't idled by the fp8 operand
                # preload, which queues behind the fp16 one on the DMA queue.
                # The 3-ahead fp16 group is emitted after this group's DR
                # matmuls: its PSUM slot frees only once this group's PSUM
                # combine has run, which itself needs the DR result.
                if g == 0:
                    for ahead in range(3):
                        ps_ms[ahead] = emit_f(xh_t, ahead)
                ps_m = ps_ms.pop(g)
                ps_c = emit_d(x8_t, gi)
                if g + 3 < G:
                    ps_ms[g + 3] = emit_f(xh_t, g + 3)
            else:
                ps_m = emit_f(xh_t, gi)
                ps_c = emit_d(x8_t, gi)
            onem, bcur = emit_feeds(ps_m, ps_c)
            pending.append((onem, bcur, st_t, gi, gb))
            if len(pending) > PIPE:
                po, pb, pst, pgi, pgb = pending.pop(0)
                emit_state(po, pb, pst, pgi)
                if pgi == G - 1:  # finished writing batch pgb's st tile
                    nc.sync.dma_start(out=spk_r[pgb], in_=pst[:])
        for po, pb, pst, pgi, pgb in pending:
            emit_state(po, pb, pst, pgi)
            if pgi == G - 1:
                nc.sync.dma_start(out=spk_r[pgb], in_=pst[:])

    _slim_waits(nc)
    _split_waits(nc)
    return nc


def _build_nc_fp16x2():
    """Previous-generation kernel: pure output sharding, fp16x2 3-pass GEMM.

    Kept for A/B timing. O_S8 = 256 outputs per core, x replicated.
    """
    from contextlib import ExitStack

    import concourse.bass as bass
    import concourse.mybir as mybir
    import concourse.tile as tile

    f32 = mybir.dt.float32
    dt_mm = mybir.dt.float16
    O_S8 = NO // NCORES
    KC8 = NI // 128

    nc = bass.Bass()
    n_planes = 2
    xT = nc.declare_dram_parameter("xT", [n_planes, NI, T * B], dt_mm, isOutput=False)
    WTs = nc.declare_dram_parameter("WTs", [n_planes, NI, O_S8], dt_mm, isOutput=False)
    ob = nc.declare_dram_parameter(
        "ob", [1, 128 + n_planes * O_S8], dt_mm, isOutput=False
    )
    spk = nc.declare_dram_parameter("spk", [T, B, O_S8], f32, isOutput=True)

    TQ = 4
    with tile.TileContext(nc) as tc, ExitStack() as ctx:
        singles = ctx.enter_context(tc.tile_pool(name="singles", bufs=1))
        xpool = ctx.enter_context(tc.tile_pool(name="xp", bufs=2))
        spool = ctx.enter_context(tc.tile_pool(name="sp", bufs=3))
        sbpool = ctx.enter_context(tc.tile_pool(name="sb", bufs=2))
        psum = ctx.enter_context(tc.tile_pool(name="ps", bufs=6, space="PSUM"))

        xTr = xT[:].rearrange("h (k p) tb -> p h k tb", p=128)
        xt0 = xpool.tile([128, n_planes, KC8, TQ * B], dt_mm)
        nc.sync.dma_start(out=xt0[:], in_=xTr[:, :, :, : TQ * B])
        wt_sb = singles.tile([128, n_planes, KC8, O_S8], dt_mm)
        WTr = WTs[:].rearrange("h (k p) o -> p h k o", p=128)
        for h in range(n_planes):
            nc.sync.dma_start(out=wt_sb[:, h], in_=WTr[:, h])
        ob_sb = singles.tile([1, 128 + n_planes * O_S8], dt_mm)
        nc.sync.dma_start(out=ob_sb[:], in_=ob[:])

        m_sb = singles.tile([128, O_S8], f32)
        w_sb = singles.tile([128, O_S8], f32)
        bias_full = singles.tile([128, O_S8], f32)
        ps_b = psum.tile([128, O_S8], f32, tag="c")
        for h in range(n_planes):
            nc.tensor.matmul(
                ps_b[:],
                lhsT=ob_sb[:, :128],
                rhs=ob_sb[:, 128 + h * O_S8 : 128 + (h + 1) * O_S8],
                start=(h == 0),
                stop=(h == n_planes - 1),
            )
        nc.vector.tensor_copy(bias_full[:], ps_b[:])
        nc.vector.tensor_copy(w_sb[:], bias_full[:])

        spk_r = spk[:].rearrange("(tq tt) b o -> tq b tt o", tt=TQ)

        for tq in range(T // TQ):
            if tq == 0:
                xt = xt0
            else:
                xt = xpool.tile([128, n_planes, KC8, TQ * B], dt_mm)
                nc.sync.dma_start(
                    out=xt[:], in_=xTr[:, :, :, tq * TQ * B : (tq + 1) * TQ * B]
                )
            st = spool.tile([128, TQ, O_S8], f32)

            for tt in range(TQ):
                ps = psum.tile([128, O_S8], f32, tag="c")
                passes = ((0, 0), (0, 1), (1, 0))
                mms = [(k, hx, hw) for k in range(KC8) for hx, hw in passes]
                for i, (k, hx, hw) in enumerate(mms):
                    nc.tensor.matmul(
                        ps[:],
                        lhsT=xt[:, hx, k, tt * B : (tt + 1) * B],
                        rhs=wt_sb[:, hw, k, :],
                        start=(i == 0),
                        stop=(i == len(mms) - 1),
                    )
                nc.vector.tensor_tensor(m_sb[:], w_sb[:], ps[:], mybir.AluOpType.add)
                nc.vector.tensor_scalar(
                    st[:, tt, :], m_sb[:], 1.0, None, mybir.AluOpType.is_gt
                )
                sb = sbpool.tile([128, O_S8], f32)
                nc.vector.tensor_tensor(
                    sb[:], st[:, tt, :], bias_full[:], mybir.AluOpType.subtract
                )
                nc.vector.scalar_tensor_tensor(
                    w_sb[:],
                    m_sb[:],
                    BETA,
                    sb[:],
                    mybir.AluOpType.mult,
                    mybir.AluOpType.subtract,
                )
            nc.sync.dma_start(out=spk_r[tq], in_=st[:])

    _slim_waits(nc)
    _split_waits(nc)
    return nc


def _build_nc(mode):
    if mode == "dr8":
        return _build_nc_dr8()
    if mode == "dr":
        return _build_nc_dr()
    return _build_nc_fp16x2()


def _slim_waits(nc):
    """Drop sync waits already implied by earlier ones (transitive closure).

    Each engine queue dispatches in FIFO order, so a wait satisfied on an
    earlier instruction of the same queue covers later instructions. A wait
    on sem s >= v also imports everything the incrementing instruction's
    queue had itself waited for when it raised s to v (semaphore vector
    clocks with snapshots at each increment).
    """
    FRAMEWORK_OPS = ("InstEventSemaphore", "InstDrain")
    engine_clock = {}  # engine -> {sem_id: value known reached}
    totals = {}  # sem_id -> running total of increments
    snapshots = {}  # sem_id -> [(value, clock dict)] in increasing value order
    poisoned = set()  # sems touched by non-monotonic updates (barriers)

    def join(dst, src):
        for s, v in src.items():
            if s in poisoned:
                continue
            if dst.get(s, -1) < v:
                dst[s] = v

    for blk in nc.m.functions[0].blocks:
        for inst in blk.instructions:
            si = getattr(inst, "sync_info", None)
            if si is None:
                continue
            is_framework = type(inst).__name__ in FRAMEWORK_OPS
            clock = engine_clock.setdefault(inst.engine, {})
            if si.on_wait:
                kept = []
                for w in si.on_wait:
                    if (
                        w.sync_type != "semaphore"
                        or w.wait_mode != "sem-ge-imm"
                        or w.id in poisoned
                    ):
                        kept.append(w)
                        continue
                    covered = clock.get(w.id, -1) >= w.wait_value
                    for val, snap in snapshots.get(w.id, ()):
                        if val <= w.wait_value:
                            join(clock, snap)
                        else:
                            break
                    if clock.get(w.id, -1) < w.wait_value:
                        clock[w.id] = w.wait_value
                    if is_framework or not covered:
                        kept.append(w)
                si.on_wait = kept
            if si.on_update:
                for u in si.on_update:
                    if u.sync_type != "semaphore":
                        continue
                    if u.update_mode not in ("sem-inc", "sem-add-imm"):
                        # barrier-style sem: stop reasoning about it entirely
                        poisoned.add(u.id)
                        totals.pop(u.id, None)
                        snapshots.pop(u.id, None)
                        for c in engine_clock.values():
                            c.pop(u.id, None)
                        continue
                    if u.id in poisoned:
                        continue
                    tot = totals.get(u.id, 0) + (u.update_value or 1)
                    totals[u.id] = tot
                    snap = dict(clock)
                    snap[u.id] = tot
                    snapshots.setdefault(u.id, []).append((tot, snap))


def _split_waits(nc, limit=1):
    """Move excess sync waits onto injected same-queue NoOps.

    Walrus codegen accepts at most `limit` sync-wait commands per engine
    instruction on this target. Engine queues dispatch in order, so a
    preceding NoOp carrying the wait is equivalent.
    """
    import concourse.mybir as mybir

    n_nops = 0
    for blk in nc.m.functions[0].blocks:
        out = []
        changed = False
        for inst in blk.instructions:
            si = getattr(inst, "sync_info", None)
            if type(inst).__name__ == "InstEventSemaphore":
                out.append(inst)
                continue
            if si is not None and si.on_wait and len(si.on_wait) > limit:
                waits = list(si.on_wait)
                for w in waits[:-limit]:
                    nop = mybir.InstNoOp(name=f"wnop-{n_nops}", ins=[], outs=[])
                    n_nops += 1
                    nop.engine = inst.engine
                    nop.sync_info = mybir.SyncInfo(on_wait=[w], on_update=[])
                    nop.bass_nofuse = True
                    out.append(nop)
                    changed = True
                si.on_wait = waits[-limit:]
            out.append(inst)
        if changed:
            try:
                blk.instructions = out
            except Exception:
                blk.instructions.clear()
                blk.instructions.extend(out)


def _split16(a):
    hi = a.astype(np.float16)
    lo = (a - hi.astype(np.float32)).astype(np.float16)
    return hi, lo


def _prepare_in_maps_dr(x, W, b):
    import ml_dtypes

    f8 = ml_dtypes.float8_e4m3
    x = np.ascontiguousarray(x, dtype=np.float32)
    W = np.ascontiguousarray(W, dtype=np.float32)
    b = np.ascontiguousarray(b, dtype=np.float32)

    # per batch group: xh [NI, T*B_S] fp16, x8 [2, NI, T*B_S] fp8
    xh_bg, x8_bg = [], []
    for bg in range(BGN):
        xc = x[:, bg * B_S : (bg + 1) * B_S, :].reshape(T * B_S, NI)
        xT = np.ascontiguousarray(xc.T)
        xh = xT.astype(np.float16)
        xl = xT - xh.astype(np.float32)
        x8 = np.empty((2, NI, T * B_S), f8)
        x8[0] = (xh.astype(np.float32) * 2.0**4).astype(f8)
        x8[1] = (xl * 2.0**16).astype(f8)
        xh_bg.append(xh)
        x8_bg.append(x8)

    # per output group: wh [NI, O_S] fp16, w8 [2, NI, O_S] fp8, bias tiles
    wh_og, w8_og, bt_og = [], [], []
    for og in range(OGN):
        Wc = W[og * O_S : (og + 1) * O_S, :]
        WT = np.ascontiguousarray(Wc.T)  # [NI, O_S]
        Wh = WT.astype(np.float16)
        Wl = WT - Wh.astype(np.float32)
        w8 = np.empty((2, NI, O_S), f8)
        w8[0] = (Wl * 2.0**19).astype(f8)  # pairs with xh8 (plane 0)
        w8[1] = (Wh.astype(np.float32) * 2.0**7).astype(f8)  # pairs with xl8
        bc = b[og * O_S : (og + 1) * O_S]
        bt = np.empty((2, 128, O_S), np.float32)
        bt[0] = 1.0 - bc
        bt[1] = BETA * bc
        wh_og.append(Wh)
        w8_og.append(w8)
        bt_og.append(bt)

    in_maps = []
    for c in range(NCORES):
        bg, og = divmod(c, OGN)
        in_maps.append(
            {
                "xh": xh_bg[bg],
                "x8": x8_bg[bg],
                "wh": wh_og[og],
                "w8": w8_og[og],
                "bt": bt_og[og],
            }
        )
    return in_maps


def _prepare_in_maps_fp16x2(x, W, b):
    O_S8 = NO // NCORES
    x = np.ascontiguousarray(x, dtype=np.float32)
    W = np.ascontiguousarray(W, dtype=np.float32)
    b = np.ascontiguousarray(b, dtype=np.float32)
    x2 = x.reshape(T * B, NI)
    xh, xl = _split16(x2)
    xT = np.stack([np.ascontiguousarray(xh.T), np.ascontiguousarray(xl.T)])
    Wh, Wl = _split16(W)
    WTs_full = np.stack([np.ascontiguousarray(Wh.T), np.ascontiguousarray(Wl.T)])
    bh, bl = _split16(b)
    b_planes = [bh, bl]
    n_planes = 2
    in_maps = []
    for c in range(NCORES):
        ob = np.empty((1, 128 + n_planes * O_S8), np.float16)
        ob[0, :128] = 1.0
        for h in range(n_planes):
            ob[0, 128 + h * O_S8 : 128 + (h + 1) * O_S8] = b_planes[h][
                c * O_S8 : (c + 1) * O_S8
            ]
        in_maps.append(
            {
                "xT": xT,
                "WTs": np.ascontiguousarray(WTs_full[:, :, c * O_S8 : (c + 1) * O_S8]),
                "ob": ob,
            }
        )
    return in_maps


def run(x, W, b, trace=False):
    """Run the kernel; returns (out [T,B,NO] fp32, BassKernelResults)."""
    from concourse.bass_utils import run_bass_kernel_spmd

    if MODE not in _cache:
        _cache[MODE] = _build_nc(MODE)
    nc = _cache[MODE]
    if MODE == "dr8":
        in_maps = _prepare_in_maps_dr8(x, W, b)
    elif MODE == "dr":
        in_maps = _prepare_in_maps_dr(x, W, b)
    else:
        in_maps = _prepare_in_maps_fp16x2(x, W, b)
    res = run_bass_kernel_spmd(nc, in_maps, list(range(NCORES)), trace=trace)
    if MODE == "dr8":
        out = np.empty((T, B, NO), np.float32)
        for c in range(NCORES):
            out[:, :, c * O_S8 : (c + 1) * O_S8] = res.results[c]["spk"].astype(
                np.float32
            )
    elif MODE == "dr":
        out = np.empty((T, B, NO), np.float32)
        for c in range(NCORES):
            bg, og = divmod(c, OGN)
            out[:, bg * B_S : (bg + 1) * B_S, og * O_S : (og + 1) * O_S] = res.results[
                c
            ]["spk"].astype(np.float32)
    else:
        out = np.concatenate([res.results[c]["spk"] for c in range(NCORES)], axis=2)
    return out, res


def kernel(x, W, b):
    out, _ = run(x, W, b, trace=False)
    return out


# revision 61
# speedup vs baseline: 1.0078x; 1.0000x over previous
"""Trainium2 Bass kernel for nn_FFNet_17600775979626.

Spiking FFN layer: cur = einsum('tbi,oi->tbo', x, W) + b, followed by a
leaky-integrate-and-fire scan over T with subtractive reset (snntorch Leaky,
beta=0.95, threshold=1.0). Returns spk_rec [T, B, NO] (0.0/1.0 floats).

MODE "dr8" (default) — fp16 + fp8-DoubleRow GEMM, output sharding:

  Each core owns a 256-wide slice of the 2048 outputs and all of (T, B).
  GEMM precision scheme (validated vs fp64: cur err std ~1e-5, ~250 spike
  mismatches of 33.5M -> rel err 8.8e-3):
    x = xh + xl, W = Wh + Wl  (fp16 hi + fp32 residual)
    cur ~= xh@Wh                                   # fp16 matmuls, 1 cyc/row
         + 2^-21 * (xh8@Wl8 + xl8@Wh8)             # one fp8 DoubleRow matmul
    where xh8 = fp8(xh), xl8 = fp8(xl*2^16), Wl8 = fp8(Wl*2^21),
    Wh8 = fp8(Wh*2^5). A DoubleRow matmul contracts BOTH correction plane
    pairs in a single pass at 0.5 cyc/row, so a k-chunk costs 256+128 PE
    cycles instead of 3*256 (fp16x2 baseline) — a 2x tensor-engine speedup.
    The dropped xl@Wl term is ~2^-22 relative. See _build_nc_dr8 for the
    dataflow (on-chip xh8 cast, scan structure, engine assignment).

  Scan reformulation (bias folded into per-step feed tensors):
      curb_t = cur_t + b        onem_t = 1 - curb_t     bcur_t = beta*curb_t
      st_t   = w_{t-1} > onem_t
      w_t    = (beta*w_{t-1} + bcur_t) - st_t

MODE "dr" — same GEMM scheme on a 2 (batch) x 4 (output) core grid with two
timesteps per matmul group. REJECTED by the walrus birverifier: its [64, 512]
half-partition scan ops violate the equal-base-partition rule for SBUF ALU
operands. Kept for reference; do not ship.

Walrus codegen on this target accepts at most ONE sync-wait command per
engine instruction, while Tile's wait assigner freely emits several. Two
post-scheduling passes fix that: _slim_waits drops waits already implied
transitively (per-queue FIFO dispatch + semaphore vector clocks), and
_split_waits moves any excess waits onto injected same-queue NoOps.
"""

import os

import numpy as np

T, B, NI, NO = 128, 128, 2048, 2048
NCORES = 8
BGN, OGN = 2, 4  # batch groups x output groups
B_S = B // BGN  # 64 batch rows per core
O_S = NO // OGN  # 512 output neurons per core
KC = NI // 128  # 16 contraction chunks
BETA = 0.95
G = 4  # M-groups (2 steps each) per DMA batch
NG = T * B_S // 128  # 64 M-groups (= T/2)
CSH = 23  # fp8 correction PSUM carries 2^CSH * (xh@Wl + xl@Wh)

MODE = os.environ.get("KERNEL_MODE", "dr8")

_cache = {}

O_S8 = NO // NCORES  # 256 output neurons per core in dr8 mode
TQ8 = 8  # timesteps per DMA batch in dr8 mode
CSH8 = 21  # dr8: correction PSUM carries 2^CSH8 * (xh@Wl + xl@Wh)


def _build_nc_dr8():
    """1x8 output sharding, fp16 + fp8-DoubleRow GEMM, one timestep per
    matmul group.

    Each core owns a 256-wide output slice and all of (T, B). Per step the
    GEMM is 16 fp16 k-chunk matmuls (xh@Wh -> ps_m) plus 16 fp8 DoubleRow
    matmuls contracting both correction plane pairs at 0.5 cyc/row
    (xh8@Wl8 + xl8@Wh8 -> ps_c, scales 2^0*2^21 and 2^16*2^5 = 2^21).
    x ships as fp16 (xh) + one fp8 plane (xl8): xl8 is DMAed straight into
    plane 1 of the per-batch x8c tile, and the xh8 plane is derived on-chip
    (the Act engine does a pure fp16->fp8 copy into plane 0, two steps ahead
    of the PE). That keeps per-core DMA at ~104MB, under the ~340GB/s DMA
    pool at the PE's 327us floor. DMA transfers serialize on a shared engine
    pool in issue order, so batches move as half-batch chunks interleaved in
    consumption order and compute gates on subtile completion sems.

    The LIF scan runs at full width [128, 256] (batch on partitions), so
    every elementwise op has base partition 0 (the walrus birverifier
    rejects SBUF ALU operands with differing base partitions). All six scan
    ops sit on DVE (~2.0us/step < 2.56us PE budget), so any serialization
    the Tile scheduler picks still fits; the PSUM downscale runs on Act.
    """
    from contextlib import ExitStack

    import concourse.bass as bass
    import concourse.mybir as mybir
    import concourse.tile as tile

    f32 = mybir.dt.float32
    f16 = mybir.dt.float16
    f8 = mybir.dt.float8e4
    DR = mybir.MatmulPerfMode.DoubleRow
    TB = T * B
    TQ = TQ8
    NB = T // TQ

    nc = bass.Bass()
    xh = nc.declare_dram_parameter("xh", [NI, TB], f16, isOutput=False)
    xl8 = nc.declare_dram_parameter("xl8", [NI, TB], f8, isOutput=False)
    wh = nc.declare_dram_parameter("wh", [NI, O_S8], f16, isOutput=False)
    w8l = nc.declare_dram_parameter("w8l", [NI, O_S8], f8, isOutput=False)
    # plane 0: 1 - b (bcast over partitions); plane 1: beta * b
    bt = nc.declare_dram_parameter("bt", [2, 128, O_S8], f32, isOutput=False)
    spk = nc.declare_dram_parameter("spk", [T, B, O_S8], f16, isOutput=True)

    with tile.TileContext(nc) as tc, ExitStack() as ctx:
        singles = ctx.enter_context(tc.tile_pool(name="singles", bufs=1))
        xhp = ctx.enter_context(tc.tile_pool(name="xhp", bufs=2))
        x8cp = ctx.enter_context(tc.tile_pool(name="x8cp", bufs=2))
        stp = ctx.enter_context(tc.tile_pool(name="stp", bufs=2))
        scr = ctx.enter_context(tc.tile_pool(name="scr", bufs=3))
        pmp = ctx.enter_context(tc.tile_pool(name="pmp", bufs=4, space="PSUM"))
        pcp = ctx.enter_context(tc.tile_pool(name="pcp", bufs=4, space="PSUM"))

        xhr = xh[:].rearrange("(k p) tb -> p k tb", p=128)
        xlr = xl8[:].rearrange("(k p) tb -> p k tb", p=128)

        # The x8c batch tile pairs the DoubleRow planes: plane 1 (xl8) is
        # DMAed straight from DRAM; plane 0 (fp8(xh)) is filled per step by
        # an Act-engine fp16->fp8 copy. No on-chip plane shuffling needed.
        # The DMA engines are a shared-bandwidth pool and transfers serialize
        # in issue order, so the preload interleaves operand kinds: the fp16
        # side of the first half-batch lands first (fp16 matmuls start ~9us
        # in), then the fp8 side (DoubleRow groups join at ~12us).
        xh_ts = {0: xhp.tile([128, KC, TQ * 128], f16, name="xh_t")}
        x8c_ts = {0: x8cp.tile([128, 2, KC, TQ * 128], f8, name="x8c_t")}
        wh_sb = singles.tile([128, KC, O_S8], f16)
        w8_sb = singles.tile([128, 2, KC, O_S8], f8)
        bias_sb = singles.tile([128, 2, O_S8], f32)
        h = TQ * 64  # 4-timestep column chunk
        q = TQ * 32  # 2-timestep column chunk
        nc.sync.dma_start(out=xh_ts[0][:, :, :h], in_=xhr[:, :, :h])
        nc.sync.dma_start(out=wh_sb[:], in_=wh[:].rearrange("(k p) o -> p k o", p=128))
        nc.sync.dma_start(out=x8c_ts[0][:, 1, :, :h], in_=xlr[:, :, :h])
        nc.sync.dma_start(
            out=w8_sb[:, 0], in_=w8l[:].rearrange("(k p) o -> p k o", p=128)
        )
        nc.sync.dma_start(out=xh_ts[0][:, :, h:], in_=xhr[:, :, h : TQ * 128])
        nc.sync.dma_start(out=x8c_ts[0][:, 1, :, h:], in_=xlr[:, :, h : TQ * 128])
        # bias is only needed by the scan (~16us in, with slack that
        # self-heals), so it yields its queue slot to the PE-feeding halves
        nc.sync.dma_start(out=bias_sb[:], in_=bt[:].rearrange("h p o -> p h o"))

        w_sb = singles.tile([128, O_S8], f32)  # carry: beta*m + b - spk
        nc.vector.memset(w_sb[:], 0.0)

        spk_r = spk[:].rearrange("(gb ti) b o -> gb b ti o", ti=TQ)

        def ensure_batch(gb):
            # Halved, consumption-ordered transfers: matmuls gate on subtile
            # completion sems, so the batch's first 4 steps can start while
            # its second half is still in flight.
            if gb in xh_ts or gb >= NB:
                return
            xh_t = xhp.tile([128, KC, TQ * 128], f16, name="xh_t")
            x8c_t = x8cp.tile([128, 2, KC, TQ * 128], f8, name="x8c_t")
            base = gb * TQ * 128
            for lo, hi in ((0, h), (h, TQ * 128)):
                nc.sync.dma_start(
                    out=xh_t[:, :, lo:hi], in_=xhr[:, :, base + lo : base + hi]
                )
                nc.sync.dma_start(
                    out=x8c_t[:, 1, :, lo:hi], in_=xlr[:, :, base + lo : base + hi]
                )
            xh_ts[gb], x8c_ts[gb] = xh_t, x8c_t

        def emit_cast(t):
            # Fill plane 0 of x8c for step t: a pure fp16->fp8 copy on Act.
            if t >= T:
                return
            gb, ti = divmod(t, TQ)
            cw = slice(ti * 128, (ti + 1) * 128)
            nc.scalar.activation(
                x8c_ts[gb][:, 0, :, cw],
                xh_ts[gb][:, :, cw],
                mybir.ActivationFunctionType.Copy,
            )

        st_ts = {}
        emit_cast(0)
        emit_cast(1)
        # Wh8 plane of the DoubleRow weights is derived on-chip from the
        # fp16 Wh (same Act cast as the x plane, scale 2^5), saving 0.5MB of
        # preload traffic. Two halves so early DR k-chunks start sooner.
        for lo, hi in ((0, KC // 2), (KC // 2, KC)):
            nc.scalar.activation(
                w8_sb[:, 1, lo:hi, :],
                wh_sb[:, lo:hi, :],
                mybir.ActivationFunctionType.Copy,
                scale=2.0**5,
            )
        for t in range(T):
            gb, ti = divmod(t, TQ)
            if ti == 0:
                st_ts[gb] = stp.tile([128, TQ, O_S8], f16, name="st_t")
            if ti == 0:
                ensure_batch(gb + 1)

            ps_m = pmp.tile([128, O_S8], f32, tag="m")
            cw = slice(ti * 128, (ti + 1) * 128)
            for k in range(KC):
                nc.tensor.matmul(
                    ps_m[:],
                    lhsT=xh_ts[gb][:, k, cw],
                    rhs=wh_sb[:, k, :],
                    start=(k == 0),
                    stop=(k == KC - 1),
                )
            ps_c = pcp.tile([128, O_S8], f32, tag="c")
            for k in range(KC):
                nc.tensor.matmul(
                    ps_c[:],
                    lhsT=x8c_ts[gb][:, :, k, cw],
                    rhs=w8_sb[:, :, k, :],
                    start=(k == 0),
                    stop=(k == KC - 1),
                    perf_mode=DR,
                )
            emit_cast(t + 2)

            # scan: curb = ps_m + 2^-CSH8*ps_c + b (bias folded into the
            # onem/bcur tiles); st = w > 1-curb; w' = (beta*w + beta*curb)-st
            c1 = scr.tile([128, O_S8], f32, tag="c1")
            nc.scalar.activation(
                c1[:], ps_c[:], mybir.ActivationFunctionType.Copy, scale=2.0**-CSH8
            )
            c0 = scr.tile([128, O_S8], f32, tag="c0")
            nc.vector.tensor_tensor(c0[:], c1[:], ps_m[:], mybir.AluOpType.add)
            onem = scr.tile([128, O_S8], f32, tag="onem")
            nc.vector.scalar_tensor_tensor(
                onem[:],
                c0[:],
                -1.0,
                bias_sb[:, 0, :],
                mybir.AluOpType.mult,
                mybir.AluOpType.add,
            )
            bcur = scr.tile([128, O_S8], f32, tag="bcur")
            nc.vector.scalar_tensor_tensor(
                bcur[:],
                c0[:],
                BETA,
                bias_sb[:, 1, :],
                mybir.AluOpType.mult,
                mybir.AluOpType.add,
            )
            stv = st_ts[gb][:, ti, :]
            nc.vector.tensor_tensor(stv, w_sb[:], onem[:], mybir.AluOpType.is_gt)
            if t < T - 1:  # the final carry update is dead code
                p_t = scr.tile([128, O_S8], f32, tag="p")
                nc.vector.scalar_tensor_tensor(
                    p_t[:],
                    w_sb[:],
                    BETA,
                    bcur[:],
                    mybir.AluOpType.mult,
                    mybir.AluOpType.add,
                )
                nc.vector.tensor_tensor(w_sb[:], p_t[:], stv, mybir.AluOpType.subtract)

            # spikes leave in half-batches (quarters at the very end):
            # keeps the out-queue smooth and shortens the final drain tail.
            if ti == TQ // 2 - 1:
                nc.sync.dma_start(
                    out=spk_r[gb, :, : TQ // 2], in_=st_ts[gb][:, : TQ // 2, :]
                )
            elif gb == NB - 1 and ti == 5:
                nc.sync.dma_start(out=spk_r[gb, :, 4:6], in_=st_ts[gb][:, 4:6, :])
            elif ti == TQ - 1:
                lo = 6 if gb == NB - 1 else TQ // 2
                nc.sync.dma_start(
                    out=spk_r[gb, :, lo:], in_=st_ts[gb][:, lo:, :]
                )

    _slim_waits(nc)
    _split_waits(nc)
    return nc


def _prepare_in_maps_dr8(x, W, b):
    import ml_dtypes

    f8 = ml_dtypes.float8_e4m3
    x = np.ascontiguousarray(x, dtype=np.float32)
    W = np.ascontiguousarray(W, dtype=np.float32)
    b = np.ascontiguousarray(b, dtype=np.float32)

    x2 = x.reshape(T * B, NI)
    xT = np.ascontiguousarray(x2.T)
    xh = xT.astype(np.float16)
    xl8 = ((xT - xh.astype(np.float32)) * 2.0**16).astype(f8)

    in_maps = []
    for c in range(NCORES):
        Wc = W[c * O_S8 : (c + 1) * O_S8, :]
        WT = np.ascontiguousarray(Wc.T)  # [NI, O_S8]
        Wh = WT.astype(np.float16)
        Wl = WT - Wh.astype(np.float32)
        w8l = (Wl * 2.0**21).astype(f8)  # pairs with fp8(xh) (plane 0)
        bc = b[c * O_S8 : (c + 1) * O_S8]
        bt = np.empty((2, 128, O_S8), np.float32)
        bt[0] = 1.0 - bc
        bt[1] = BETA * bc
        in_maps.append({"xh": xh, "xl8": xl8, "wh": Wh, "w8l": w8l, "bt": bt})
    return in_maps


def _build_nc_dr():
    from contextlib import ExitStack

    import concourse.bass as bass
    import concourse.mybir as mybir
    import concourse.tile as tile

    f32 = mybir.dt.float32
    f16 = mybir.dt.float16
    f8 = mybir.dt.float8e4
    DR = mybir.MatmulPerfMode.DoubleRow
    TB = T * B_S

    nc = bass.Bass()
    xh = nc.declare_dram_parameter("xh", [NI, TB], f16, isOutput=False)
    x8 = nc.declare_dram_parameter("x8", [2, NI, TB], f8, isOutput=False)
    wh = nc.declare_dram_parameter("wh", [NI, O_S], f16, isOutput=False)
    w8 = nc.declare_dram_parameter("w8", [2, NI, O_S], f8, isOutput=False)
    # plane 0: 1 - b (bcast over partitions); plane 1: beta * b
    bt = nc.declare_dram_parameter("bt", [2, 128, O_S], f32, isOutput=False)
    spk = nc.declare_dram_parameter("spk", [T, B_S, O_S], f16, isOutput=True)

    with tile.TileContext(nc) as tc, ExitStack() as ctx:
        singles = ctx.enter_context(tc.tile_pool(name="singles", bufs=1))
        xhp = ctx.enter_context(tc.tile_pool(name="xhp", bufs=2))
        x8p = ctx.enter_context(tc.tile_pool(name="x8p", bufs=2))
        stp = ctx.enter_context(tc.tile_pool(name="stp", bufs=2))
        scr = ctx.enter_context(tc.tile_pool(name="scr", bufs=2))
        pmp = ctx.enter_context(tc.tile_pool(name="pmp", bufs=3, space="PSUM"))
        pcp = ctx.enter_context(tc.tile_pool(name="pcp", bufs=4, space="PSUM"))

        xhr = xh[:].rearrange("(k p) tb -> p k tb", p=128)
        x8r = x8[:].rearrange("h (k p) tb -> p h k tb", p=128)

        # DMA issue order sets arrival order on the queue: the fp16 operands
        # (xh batch 0, Wh) first so pass-1 matmuls start ~11us in, then the
        # fp8 operands for the DoubleRow groups.
        xh_t0 = xhp.tile([128, KC, G * 128], f16)
        nc.sync.dma_start(out=xh_t0[:], in_=xhr[:, :, : G * 128])
        wh_sb = singles.tile([128, KC, O_S], f16)
        nc.sync.dma_start(out=wh_sb[:], in_=wh[:].rearrange("(k p) o -> p k o", p=128))
        x8_t0 = x8p.tile([128, 2, KC, G * 128], f8)
        nc.sync.dma_start(out=x8_t0[:], in_=x8r[:, :, :, : G * 128])
        w8_sb = singles.tile([128, 2, KC, O_S], f8)
        nc.sync.dma_start(
            out=w8_sb[:, 0], in_=w8l[:].rearrange("(k p) o -> p k o", p=128)
        )
        bias_sb = singles.tile([128, 2, O_S], f32)
        nc.sync.dma_start(out=bias_sb[:], in_=bt[:].rearrange("h p o -> p h o"))

        w_sb = singles.tile([64, O_S], f32)  # carry: beta*m - spk, per (b, o)
        nc.vector.memset(w_sb[:], 0.0)

        spk_r = spk[:].rearrange("(gb gi s) b o -> gb (s b) gi o", gi=G, s=2)

        def emit_f(xh_t, gi):
            ps_m = pmp.tile([128, O_S], f32, tag="m")
            cw = slice(gi * 128, (gi + 1) * 128)
            for k in range(KC):
                nc.tensor.matmul(
                    ps_m[:],
                    lhsT=xh_t[:, k, cw],
                    rhs=wh_sb[:, k, :],
                    start=(k == 0),
                    stop=(k == KC - 1),
                )
            return ps_m

        def emit_d(x8_t, gi):
            ps_c = pcp.tile([128, O_S], f32, tag="c")
            cw = slice(gi * 128, (gi + 1) * 128)
            for k in range(KC):
                nc.tensor.matmul(
                    ps_c[:],
                    lhsT=x8_t[:, :, k, cw],
                    rhs=w8_sb[:, :, k, :],
                    start=(k == 0),
                    stop=(k == KC - 1),
                    perf_mode=DR,
                )
            return ps_c

        def emit_feeds(ps_m, ps_c):
            # curb = ps_m + 2^-CSH*ps_c + b, then the bias-folded scan
            # tensors; stt ops cannot take two PSUM sources, so the otherwise
            # idle Act engine downscales the correction PSUM.
            c1 = scr.tile([128, O_S], f32, tag="c1", bufs=3)
            nc.scalar.activation(
                c1[:], ps_c[:], mybir.ActivationFunctionType.Copy, scale=2.0**-CSH
            )
            c0 = scr.tile([128, O_S], f32, tag="c0", bufs=3)
            nc.vector.tensor_tensor(c0[:], c1[:], ps_m[:], mybir.AluOpType.add)
            onem = scr.tile([128, O_S], f32, tag="onem", bufs=3)
            nc.gpsimd.scalar_tensor_tensor(
                onem[:],
                c0[:],
                -1.0,
                bias_sb[:, 0, :],
                mybir.AluOpType.mult,
                mybir.AluOpType.add,
            )
            bcur = scr.tile([128, O_S], f32, tag="bcur", bufs=3)
            nc.gpsimd.scalar_tensor_tensor(
                bcur[:],
                c0[:],
                BETA,
                bias_sb[:, 1, :],
                mybir.AluOpType.mult,
                mybir.AluOpType.add,
            )
            return onem, bcur

        def emit_state(onem, bcur, st_t, gi):
            for s in range(2):
                ph = slice(s * 64, (s + 1) * 64)
                stv = st_t[ph, gi, :]
                nc.vector.tensor_tensor(stv, w_sb[:], onem[ph, :], mybir.AluOpType.is_gt)
                p_t = scr.tile([64, O_S], f32, tag="p", bufs=4)
                nc.gpsimd.scalar_tensor_tensor(
                    p_t[:],
                    w_sb[:],
                    BETA,
                    bcur[ph, :],
                    mybir.AluOpType.mult,
                    mybir.AluOpType.add,
                )
                nc.vector.tensor_tensor(w_sb[:], p_t[:], stv, mybir.AluOpType.subtract)

        # The scan is software-pipelined one M-group deep: group g's feed ops
        # (PSUM combine + bias folds, no serial state dependency) are emitted
        # BEFORE group g-1's state-update ops so the engine FIFOs never force
        # the feeds behind the w-chain. The true critical cycle is then just
        # st/p -> w per sub-step (~3.4us), under the PE's 5.1us per group.
        PIPE = 2  # scan pipeline depth in M-groups
        pending = []  # deferred (onem, bcur, st_t, gi, gb) awaiting state ops
        xh_t = x8_t = st_t = None
        ps_ms = {}
        for g in range(NG):
            gb, gi = divmod(g, G)
            if gi == 0:
                if gb == 0:
                    xh_t, x8_t = xh_t0, x8_t0
                else:
                    xh_t = xhp.tile([128, KC, G * 128], f16)
                    nc.sync.dma_start(
                        out=xh_t[:], in_=xhr[:, :, gb * G * 128 : (gb + 1) * G * 128]
                    )
                    x8_t = x8p.tile([128, 2, KC, G * 128], f8)
                    nc.sync.dma_start(
                        out=x8_t[:],
                        in_=x8r[:, :, :, gb * G * 128 : (gb + 1) * G * 128],
                    )
                st_t = stp.tile([128, G, O_S], f16)
            if gb == 0:
                # Batch 0: run fp16 groups up to 3 ahead of the DoubleRow
                # groups so the PE (in-order) isn't idled by the fp8 operand
                # preload, which queues behind the fp16 one on the DMA queue.
                # The 3-ahead fp16 group is emitted after this group's DR
                # matmuls: its PSUM slot frees only once this group's PSUM
                # combine has run, which itself needs the DR result.
                if g == 0:
                    for ahead in range(3):
                        ps_ms[ahead] = emit_f(xh_t, ahead)
                ps_m = ps_ms.pop(g)
                ps_c = emit_d(x8_t, gi)
                if g + 3 < G:
                    ps_ms[g + 3] = emit_f(xh_t, g + 3)
            else:
                ps_m = emit_f(xh_t, gi)
                ps_c = emit_d(x8_t, gi)
            onem, bcur = emit_feeds(ps_m, ps_c)
            pending.append((onem, bcur, st_t, gi, gb))
            if len(pending) > PIPE:
                po, pb, pst, pgi, pgb = pending.pop(0)
                emit_state(po, pb, pst, pgi)
                if pgi == G - 1:  # finished writing batch pgb's st tile
                    nc.sync.dma_start(out=spk_r[pgb], in_=pst[:])
        for po, pb, pst, pgi, pgb in pending:
            emit_state(po, pb, pst, pgi)
            if pgi == G - 1:
                nc.sync.dma_start(out=spk_r[pgb], in_=pst[:])

    _slim_waits(nc)
    _split_waits(nc)
    return nc


def _build_nc_fp16x2():
    """Previous-generation kernel: pure output sharding, fp16x2 3-pass GEMM.

    Kept for A/B timing. O_S8 = 256 outputs per core, x replicated.
    """
    from contextlib import ExitStack

    import concourse.bass as bass
    import concourse.mybir as mybir
    import concourse.tile as tile

    f32 = mybir.dt.float32
    dt_mm = mybir.dt.float16
    O_S8 = NO // NCORES
    KC8 = NI // 128

    nc = bass.Bass()
    n_planes = 2
    xT = nc.declare_dram_parameter("xT", [n_planes, NI, T * B], dt_mm, isOutput=False)
    WTs = nc.declare_dram_parameter("WTs", [n_planes, NI, O_S8], dt_mm, isOutput=False)
    ob = nc.declare_dram_parameter(
        "ob", [1, 128 + n_planes * O_S8], dt_mm, isOutput=False
    )
    spk = nc.declare_dram_parameter("spk", [T, B, O_S8], f32, isOutput=True)

    TQ = 4
    with tile.TileContext(nc) as tc, ExitStack() as ctx:
        singles = ctx.enter_context(tc.tile_pool(name="singles", bufs=1))
        xpool = ctx.enter_context(tc.tile_pool(name="xp", bufs=2))
        spool = ctx.enter_context(tc.tile_pool(name="sp", bufs=3))
        sbpool = ctx.enter_context(tc.tile_pool(name="sb", bufs=2))
        psum = ctx.enter_context(tc.tile_pool(name="ps", bufs=6, space="PSUM"))

        xTr = xT[:].rearrange("h (k p) tb -> p h k tb", p=128)
        xt0 = xpool.tile([128, n_planes, KC8, TQ * B], dt_mm)
        nc.sync.dma_start(out=xt0[:], in_=xTr[:, :, :, : TQ * B])
        wt_sb = singles.tile([128, n_planes, KC8, O_S8], dt_mm)
        WTr = WTs[:].rearrange("h (k p) o -> p h k o", p=128)
        for h in range(n_planes):
            nc.sync.dma_start(out=wt_sb[:, h], in_=WTr[:, h])
        ob_sb = singles.tile([1, 128 + n_planes * O_S8], dt_mm)
        nc.sync.dma_start(out=ob_sb[:], in_=ob[:])

        m_sb = singles.tile([128, O_S8], f32)
        w_sb = singles.tile([128, O_S8], f32)
        bias_full = singles.tile([128, O_S8], f32)
        ps_b = psum.tile([128, O_S8], f32, tag="c")
        for h in range(n_planes):
            nc.tensor.matmul(
                ps_b[:],
                lhsT=ob_sb[:, :128],
                rhs=ob_sb[:, 128 + h * O_S8 : 128 + (h + 1) * O_S8],
                start=(h == 0),
                stop=(h == n_planes - 1),
            )
        nc.vector.tensor_copy(bias_full[:], ps_b[:])
        nc.vector.tensor_copy(w_sb[:], bias_full[:])

        spk_r = spk[:].rearrange("(tq tt) b o -> tq b tt o", tt=TQ)

        for tq in range(T // TQ):
            if tq == 0:
                xt = xt0
            else:
                xt = xpool.tile([128, n_planes, KC8, TQ * B], dt_mm)
                nc.sync.dma_start(
                    out=xt[:], in_=xTr[:, :, :, tq * TQ * B : (tq + 1) * TQ * B]
                )
            st = spool.tile([128, TQ, O_S8], f32)

            for tt in range(TQ):
                ps = psum.tile([128, O_S8], f32, tag="c")
                passes = ((0, 0), (0, 1), (1, 0))
                mms = [(k, hx, hw) for k in range(KC8) for hx, hw in passes]
                for i, (k, hx, hw) in enumerate(mms):
                    nc.tensor.matmul(
                        ps[:],
                        lhsT=xt[:, hx, k, tt * B : (tt + 1) * B],
                        rhs=wt_sb[:, hw, k, :],
                        start=(i == 0),
                        stop=(i == len(mms) - 1),
                    )
                nc.vector.tensor_tensor(m_sb[:], w_sb[:], ps[:], mybir.AluOpType.add)
                nc.vector.tensor_scalar(
                    st[:, tt, :], m_sb[:], 1.0, None, mybir.AluOpType.is_gt
                )
                sb = sbpool.tile([128, O_S8], f32)
                nc.vector.tensor_tensor(
                    sb[:], st[:, tt, :], bias_full[:], mybir.AluOpType.subtract
                )
                nc.vector.scalar_tensor_tensor(
                    w_sb[:],
                    m_sb[:],
                    BETA,
                    sb[:],
                    mybir.AluOpType.mult,
                    mybir.AluOpType.subtract,
                )
            nc.sync.dma_start(out=spk_r[tq], in_=st[:])

    _slim_waits(nc)
    _split_waits(nc)
    return nc


def _build_nc(mode):
    if mode == "dr8":
        return _build_nc_dr8()
    if mode == "dr":
        return _build_nc_dr()
    return _build_nc_fp16x2()


def _slim_waits(nc):
    """Drop sync waits already implied by earlier ones (transitive closure).

    Each engine queue dispatches in FIFO order, so a wait satisfied on an
    earlier instruction of the same queue covers later instructions. A wait
    on sem s >= v also imports everything the incrementing instruction's
    queue had itself waited for when it raised s to v (semaphore vector
    clocks with snapshots at each increment).
    """
    FRAMEWORK_OPS = ("InstEventSemaphore", "InstDrain")
    engine_clock = {}  # engine -> {sem_id: value known reached}
    totals = {}  # sem_id -> running total of increments
    snapshots = {}  # sem_id -> [(value, clock dict)] in increasing value order
    poisoned = set()  # sems touched by non-monotonic updates (barriers)

    def join(dst, src):
        for s, v in src.items():
            if s in poisoned:
                continue
            if dst.get(s, -1) < v:
                dst[s] = v

    for blk in nc.m.functions[0].blocks:
        for inst in blk.instructions:
            si = getattr(inst, "sync_info", None)
            if si is None:
                continue
            is_framework = type(inst).__name__ in FRAMEWORK_OPS
            clock = engine_clock.setdefault(inst.engine, {})
            if si.on_wait:
                kept = []
                for w in si.on_wait:
                    if (
                        w.sync_type != "semaphore"
                        or w.wait_mode != "sem-ge-imm"
                        or w.id in poisoned
                    ):
                        kept.append(w)
                        continue
                    covered = clock.get(w.id, -1) >= w.wait_value
                    for val, snap in snapshots.get(w.id, ()):
                        if val <= w.wait_value:
                            join(clock, snap)
                        else:
                            break
                    if clock.get(w.id, -1) < w.wait_value:
                        clock[w.id] = w.wait_value
                    if is_framework or not covered:
                        kept.append(w)
                si.on_wait = kept
            if si.on_update:
                for u in si.on_update:
                    if u.sync_type != "semaphore":
                        continue
                    if u.update_mode not in ("sem-inc", "sem-add-imm"):
                        # barrier-style sem: stop reasoning about it entirely
                        poisoned.add(u.id)
                        totals.pop(u.id, None)
                        snapshots.pop(u.id, None)
                        for c in engine_clock.values():
                            c.pop(u.id, None)
                        continue
                    if u.id in poisoned:
                        continue
                    tot = totals.get(u.id, 0) + (u.update_value or 1)
                    totals[u.id] = tot
                    snap = dict(clock)
                    snap[u.id] = tot
                    snapshots.setdefault(u.id, []).append((tot, snap))


def _split_waits(nc, limit=1):
    """Move excess sync waits onto injected same-queue NoOps.

    Walrus codegen accepts at most `limit` sync-wait commands per engine
    instruction on this target. Engine queues dispatch in order, so a
    preceding NoOp carrying the wait is equivalent.
    """
    import concourse.mybir as mybir

    n_nops = 0
    for blk in nc.m.functions[0].blocks:
        out = []
        changed = False
        for inst in blk.instructions:
            si = getattr(inst, "sync_info", None)
            if type(inst).__name__ == "InstEventSemaphore":
                out.append(inst)
                continue
            if si is not None and si.on_wait and len(si.on_wait) > limit:
                waits = list(si.on_wait)
                for w in waits[:-limit]:
                    nop = mybir.InstNoOp(name=f"wnop-{n_nops}", ins=[], outs=[])
                    n_nops += 1
                    nop.engine = inst.engine
                    nop.sync_info = mybir.SyncInfo(on_wait=[w], on_update=[])
                    nop.bass_nofuse = True
                    out.append(nop)
                    changed = True
                si.on_wait = waits[-limit:]
            out.append(inst)
        if changed:
            try:
                blk.instructions = out
            except Exception:
                blk.instructions.clear()
                blk.instructions.extend(out)


def _split16(a):
    hi = a.astype(np.float16)
    lo = (a - hi.astype(np.float32)).astype(np.float16)
    return hi, lo


def _prepare_in_maps_dr(x, W, b):
    import ml_dtypes

    f8 = ml_dtypes.float8_e4m3
    x = np.ascontiguousarray(x, dtype=np.float32)
    W = np.ascontiguousarray(W, dtype=np.float32)
    b = np.ascontiguousarray(b, dtype=np.float32)

    # per batch group: xh [NI, T*B_S] fp16, x8 [2, NI, T*B_S] fp8
    xh_bg, x8_bg = [], []
    for bg in range(BGN):
        xc = x[:, bg * B_S : (bg + 1) * B_S, :].reshape(T * B_S, NI)
        xT = np.ascontiguousarray(xc.T)
        xh = xT.astype(np.float16)
        xl = xT - xh.astype(np.float32)
        x8 = np.empty((2, NI, T * B_S), f8)
        x8[0] = (xh.astype(np.float32) * 2.0**4).astype(f8)
        x8[1] = (xl * 2.0**16).astype(f8)
        xh_bg.append(xh)
        x8_bg.append(x8)

    # per output group: wh [NI, O_S] fp16, w8 [2, NI, O_S] fp8, bias tiles
    wh_og, w8_og, bt_og = [], [], []
    for og in range(OGN):
        Wc = W[og * O_S : (og + 1) * O_S, :]
        WT = np.ascontiguousarray(Wc.T)  # [NI, O_S]
        Wh = WT.astype(np.float16)
        Wl = WT - Wh.astype(np.float32)
        w8 = np.empty((2, NI, O_S), f8)
        w8[0] = (Wl * 2.0**19).astype(f8)  # pairs with xh8 (plane 0)
        w8[1] = (Wh.astype(np.float32) * 2.0**7).astype(f8)  # pairs with xl8
        bc = b[og * O_S : (og + 1) * O_S]
        bt = np.empty((2, 128, O_S), np.float32)
        bt[0] = 1.0 - bc
        bt[1] = BETA * bc
        wh_og.append(Wh)
        w8_og.append(w8)
        bt_og.append(bt)

    in_maps = []
    for c in range(NCORES):
        bg, og = divmod(c, OGN)
        in_maps.append(
            {
                "xh": xh_bg[bg],
                "x8": x8_bg[bg],
                "wh": wh_og[og],
                "w8": w8_og[og],
                "bt": bt_og[og],
            }
        )
    return in_maps


def _prepare_in_maps_fp16x2(x, W, b):
    O_S8 = NO // NCORES
    x = np.ascontiguousarray(x, dtype=np.float32)
    W = np.ascontiguousarray(W, dtype=np.float32)
    b = np.ascontiguousarray(b, dtype=np.float32)
    x2 = x.reshape(T * B, NI)
    xh, xl = _split16(x2)
    xT = np.stack([np.ascontiguousarray(xh.T), np.ascontiguousarray(xl.T)])
    Wh, Wl = _split16(W)
    WTs_full = np.stack([np.ascontiguousarray(Wh.T), np.ascontiguousarray(Wl.T)])
    bh, bl = _split16(b)
    b_planes = [bh, bl]
    n_planes = 2
    in_maps = []
    for c in range(NCORES):
        ob = np.empty((1, 128 + n_planes * O_S8), np.float16)
        ob[0, :128] = 1.0
        for h in range(n_planes):
            ob[0, 128 + h * O_S8 : 128 + (h + 1) * O_S8] = b_planes[h][
                c * O_S8 : (c + 1) * O_S8
            ]
        in_maps.append(
            {
                "xT": xT,
                "WTs": np.ascontiguousarray(WTs_full[:, :, c * O_S8 : (c + 1) * O_S8]),
                "ob": ob,
            }
        )
    return in_maps


def run(x, W, b, trace=False):
    """Run the kernel; returns (out [T,B,NO] fp32, BassKernelResults)."""
    from concourse.bass_utils import run_bass_kernel_spmd

    if MODE not in _cache:
        _cache[MODE] = _build_nc(MODE)
    nc = _cache[MODE]
    if MODE == "dr8":
        in_maps = _prepare_in_maps_dr8(x, W, b)
    elif MODE == "dr":
        in_maps = _prepare_in_maps_dr(x, W, b)
    else:
        in_maps = _prepare_in_maps_fp16x2(x, W, b)
    res = run_bass_kernel_spmd(nc, in_maps, list(range(NCORES)), trace=trace)
    if MODE == "dr8":
        out = np.empty((T, B, NO), np.float32)
        for c in range(NCORES):
            out[:, :, c * O_S8 : (c + 1) * O_S8] = res.results[c]["spk"].astype(
                np.float32
            )
    elif MODE == "dr":
        out = np.empty((T, B, NO), np.float32)
        for c in range(NCORES):
            bg, og = divmod(c, OGN)
            out[:, bg * B_S : (bg + 1) * B_S, og * O_S : (og + 1) * O_S] = res.results[
                c
            ]["spk"].astype(np.float32)
    else:
        out = np.concatenate([res.results[c]["spk"] for c in range(NCORES)], axis=2)
    return out, res


def kernel(x, W, b):
    out, _ = run(x, W, b, trace=False)
    return out


# revision 62
# speedup vs baseline: 1.0085x; 1.0007x over previous
"""Trainium2 Bass kernel for nn_FFNet_17600775979626.

Spiking FFN layer: cur = einsum('tbi,oi->tbo', x, W) + b, followed by a
leaky-integrate-and-fire scan over T with subtractive reset (snntorch Leaky,
beta=0.95, threshold=1.0). Returns spk_rec [T, B, NO] (0.0/1.0 floats).

MODE "dr8" (default) — fp16 + fp8-DoubleRow GEMM, output sharding:

  Each core owns a 256-wide slice of the 2048 outputs and all of (T, B).
  GEMM precision scheme (validated vs fp64: cur err std ~1e-5, ~250 spike
  mismatches of 33.5M -> rel err 8.8e-3):
    x = xh + xl, W = Wh + Wl  (fp16 hi + fp32 residual)
    cur ~= xh@Wh                                   # fp16 matmuls, 1 cyc/row
         + 2^-21 * (xh8@Wl8 + xl8@Wh8)             # one fp8 DoubleRow matmul
    where xh8 = fp8(xh), xl8 = fp8(xl*2^16), Wl8 = fp8(Wl*2^21),
    Wh8 = fp8(Wh*2^5). A DoubleRow matmul contracts BOTH correction plane
    pairs in a single pass at 0.5 cyc/row, so a k-chunk costs 256+128 PE
    cycles instead of 3*256 (fp16x2 baseline) — a 2x tensor-engine speedup.
    The dropped xl@Wl term is ~2^-22 relative. See _build_nc_dr8 for the
    dataflow (on-chip xh8 cast, scan structure, engine assignment).

  Scan reformulation (bias folded into per-step feed tensors):
      curb_t = cur_t + b        onem_t = 1 - curb_t     bcur_t = beta*curb_t
      st_t   = w_{t-1} > onem_t
      w_t    = (beta*w_{t-1} + bcur_t) - st_t

MODE "dr" — same GEMM scheme on a 2 (batch) x 4 (output) core grid with two
timesteps per matmul group. REJECTED by the walrus birverifier: its [64, 512]
half-partition scan ops violate the equal-base-partition rule for SBUF ALU
operands. Kept for reference; do not ship.

Walrus codegen on this target accepts at most ONE sync-wait command per
engine instruction, while Tile's wait assigner freely emits several. Two
post-scheduling passes fix that: _slim_waits drops waits already implied
transitively (per-queue FIFO dispatch + semaphore vector clocks), and
_split_waits moves any excess waits onto injected same-queue NoOps.
"""

import os

import numpy as np

T, B, NI, NO = 128, 128, 2048, 2048
NCORES = 8
BGN, OGN = 2, 4  # batch groups x output groups
B_S = B // BGN  # 64 batch rows per core
O_S = NO // OGN  # 512 output neurons per core
KC = NI // 128  # 16 contraction chunks
BETA = 0.95
G = 4  # M-groups (2 steps each) per DMA batch
NG = T * B_S // 128  # 64 M-groups (= T/2)
CSH = 23  # fp8 correction PSUM carries 2^CSH * (xh@Wl + xl@Wh)

MODE = os.environ.get("KERNEL_MODE", "dr8")

_cache = {}

O_S8 = NO // NCORES  # 256 output neurons per core in dr8 mode
TQ8 = 8  # timesteps per DMA batch in dr8 mode
CSH8 = 21  # dr8: correction PSUM carries 2^CSH8 * (xh@Wl + xl@Wh)


def _build_nc_dr8():
    """1x8 output sharding, fp16 + fp8-DoubleRow GEMM, one timestep per
    matmul group.

    Each core owns a 256-wide output slice and all of (T, B). Per step the
    GEMM is 16 fp16 k-chunk matmuls (xh@Wh -> ps_m) plus 16 fp8 DoubleRow
    matmuls contracting both correction plane pairs at 0.5 cyc/row
    (xh8@Wl8 + xl8@Wh8 -> ps_c, scales 2^0*2^21 and 2^16*2^5 = 2^21).
    x ships as fp16 (xh) + one fp8 plane (xl8): xl8 is DMAed straight into
    plane 1 of the per-batch x8c tile, and the xh8 plane is derived on-chip
    (the Act engine does a pure fp16->fp8 copy into plane 0, two steps ahead
    of the PE). That keeps per-core DMA at ~104MB, under the ~340GB/s DMA
    pool at the PE's 327us floor. DMA transfers serialize on a shared engine
    pool in issue order, so batches move as half-batch chunks interleaved in
    consumption order and compute gates on subtile completion sems.

    The LIF scan runs at full width [128, 256] (batch on partitions), so
    every elementwise op has base partition 0 (the walrus birverifier
    rejects SBUF ALU operands with differing base partitions). All six scan
    ops sit on DVE (~2.0us/step < 2.56us PE budget), so any serialization
    the Tile scheduler picks still fits; the PSUM downscale runs on Act.
    """
    from contextlib import ExitStack

    import concourse.bass as bass
    import concourse.mybir as mybir
    import concourse.tile as tile

    f32 = mybir.dt.float32
    f16 = mybir.dt.float16
    f8 = mybir.dt.float8e4
    DR = mybir.MatmulPerfMode.DoubleRow
    TB = T * B
    TQ = TQ8
    NB = T // TQ

    nc = bass.Bass()
    xh = nc.declare_dram_parameter("xh", [NI, TB], f16, isOutput=False)
    xl8 = nc.declare_dram_parameter("xl8", [NI, TB], f8, isOutput=False)
    wh = nc.declare_dram_parameter("wh", [NI, O_S8], f16, isOutput=False)
    w8l = nc.declare_dram_parameter("w8l", [NI, O_S8], f8, isOutput=False)
    # plane 0: 1 - b (bcast over partitions); plane 1: beta * b
    bt = nc.declare_dram_parameter("bt", [2, 128, O_S8], f32, isOutput=False)
    spk = nc.declare_dram_parameter("spk", [T, B, O_S8], f16, isOutput=True)

    with tile.TileContext(nc) as tc, ExitStack() as ctx:
        singles = ctx.enter_context(tc.tile_pool(name="singles", bufs=1))
        xhp = ctx.enter_context(tc.tile_pool(name="xhp", bufs=2))
        x8cp = ctx.enter_context(tc.tile_pool(name="x8cp", bufs=2))
        stp = ctx.enter_context(tc.tile_pool(name="stp", bufs=2))
        scr = ctx.enter_context(tc.tile_pool(name="scr", bufs=3))
        pmp = ctx.enter_context(tc.tile_pool(name="pmp", bufs=4, space="PSUM"))
        pcp = ctx.enter_context(tc.tile_pool(name="pcp", bufs=4, space="PSUM"))

        xhr = xh[:].rearrange("(k p) tb -> p k tb", p=128)
        xlr = xl8[:].rearrange("(k p) tb -> p k tb", p=128)

        # The x8c batch tile pairs the DoubleRow planes: plane 1 (xl8) is
        # DMAed straight from DRAM; plane 0 (fp8(xh)) is filled per step by
        # an Act-engine fp16->fp8 copy. No on-chip plane shuffling needed.
        # The DMA engines are a shared-bandwidth pool and transfers serialize
        # in issue order, so the preload interleaves operand kinds: the fp16
        # side of the first half-batch lands first (fp16 matmuls start ~9us
        # in), then the fp8 side (DoubleRow groups join at ~12us).
        xh_ts = {0: xhp.tile([128, KC, TQ * 128], f16, name="xh_t")}
        x8c_ts = {0: x8cp.tile([128, 2, KC, TQ * 128], f8, name="x8c_t")}
        wh_sb = singles.tile([128, KC, O_S8], f16)
        w8_sb = singles.tile([128, 2, KC, O_S8], f8)
        bias_sb = singles.tile([128, 2, O_S8], f32)
        h = TQ * 64  # 4-timestep column chunk
        q = TQ * 32  # 2-timestep column chunk
        nc.sync.dma_start(out=xh_ts[0][:, :, :h], in_=xhr[:, :, :h])
        nc.sync.dma_start(out=wh_sb[:], in_=wh[:].rearrange("(k p) o -> p k o", p=128))
        nc.sync.dma_start(out=x8c_ts[0][:, 1, :, :h], in_=xlr[:, :, :h])
        nc.sync.dma_start(
            out=w8_sb[:, 0], in_=w8l[:].rearrange("(k p) o -> p k o", p=128)
        )
        nc.sync.dma_start(out=xh_ts[0][:, :, h:], in_=xhr[:, :, h : TQ * 128])
        nc.sync.dma_start(out=x8c_ts[0][:, 1, :, h:], in_=xlr[:, :, h : TQ * 128])
        # bias is only needed by the scan (~16us in, with slack that
        # self-heals), so it yields its queue slot to the PE-feeding halves
        nc.sync.dma_start(out=bias_sb[:], in_=bt[:].rearrange("h p o -> p h o"))

        w_sb = singles.tile([128, O_S8], f32)  # carry: beta*m + b - spk
        nc.vector.memset(w_sb[:], 0.0)

        spk_r = spk[:].rearrange("(gb ti) b o -> gb b ti o", ti=TQ)

        def ensure_batch(gb):
            # Halved, consumption-ordered transfers: matmuls gate on subtile
            # completion sems, so the batch's first 4 steps can start while
            # its second half is still in flight.
            if gb in xh_ts or gb >= NB:
                return
            xh_t = xhp.tile([128, KC, TQ * 128], f16, name="xh_t")
            x8c_t = x8cp.tile([128, 2, KC, TQ * 128], f8, name="x8c_t")
            base = gb * TQ * 128
            for lo, hi in ((0, h), (h, TQ * 128)):
                nc.sync.dma_start(
                    out=xh_t[:, :, lo:hi], in_=xhr[:, :, base + lo : base + hi]
                )
                nc.sync.dma_start(
                    out=x8c_t[:, 1, :, lo:hi], in_=xlr[:, :, base + lo : base + hi]
                )
            xh_ts[gb], x8c_ts[gb] = xh_t, x8c_t

        def emit_cast(t):
            # Fill plane 0 of x8c for step t: a pure fp16->fp8 copy on Act.
            if t >= T:
                return
            gb, ti = divmod(t, TQ)
            cw = slice(ti * 128, (ti + 1) * 128)
            nc.scalar.activation(
                x8c_ts[gb][:, 0, :, cw],
                xh_ts[gb][:, :, cw],
                mybir.ActivationFunctionType.Copy,
            )

        st_ts = {}
        emit_cast(0)
        emit_cast(1)
        # Wh8 plane of the DoubleRow weights is derived on-chip from the
        # fp16 Wh (same Act cast as the x plane, scale 2^5), saving 0.5MB of
        # preload traffic. Two halves so early DR k-chunks start sooner.
        for lo, hi in ((0, KC // 2), (KC // 2, KC)):
            nc.scalar.activation(
                w8_sb[:, 1, lo:hi, :],
                wh_sb[:, lo:hi, :],
                mybir.ActivationFunctionType.Copy,
                scale=2.0**5,
            )
        for t in range(T):
            gb, ti = divmod(t, TQ)
            if ti == 0:
                st_ts[gb] = stp.tile([128, TQ, O_S8], f16, name="st_t")
            if ti == 0:
                ensure_batch(gb + 1)

            ps_m = pmp.tile([128, O_S8], f32, tag="m")
            cw = slice(ti * 128, (ti + 1) * 128)
            for k in range(KC):
                nc.tensor.matmul(
                    ps_m[:],
                    lhsT=xh_ts[gb][:, k, cw],
                    rhs=wh_sb[:, k, :],
                    start=(k == 0),
                    stop=(k == KC - 1),
                )
            ps_c = pcp.tile([128, O_S8], f32, tag="c")
            for k in range(KC):
                nc.tensor.matmul(
                    ps_c[:],
                    lhsT=x8c_ts[gb][:, :, k, cw],
                    rhs=w8_sb[:, :, k, :],
                    start=(k == 0),
                    stop=(k == KC - 1),
                    perf_mode=DR,
                )
            emit_cast(t + 2)

            # scan: curb = ps_m + 2^-CSH8*ps_c + b (bias folded into the
            # onem/bcur tiles); st = w > 1-curb; w' = (beta*w + beta*curb)-st
            c1 = scr.tile([128, O_S8], f32, tag="c1")
            nc.scalar.activation(
                c1[:], ps_c[:], mybir.ActivationFunctionType.Copy, scale=2.0**-CSH8
            )
            c0 = scr.tile([128, O_S8], f32, tag="c0")
            nc.vector.tensor_tensor(c0[:], c1[:], ps_m[:], mybir.AluOpType.add)
            onem = scr.tile([128, O_S8], f32, tag="onem")
            nc.vector.scalar_tensor_tensor(
                onem[:],
                c0[:],
                -1.0,
                bias_sb[:, 0, :],
                mybir.AluOpType.mult,
                mybir.AluOpType.add,
            )
            if t < T - 1:  # bcur only feeds the carry update; dead at t=T-1
                bcur = scr.tile([128, O_S8], f32, tag="bcur")
                nc.vector.scalar_tensor_tensor(
                    bcur[:],
                    c0[:],
                    BETA,
                    bias_sb[:, 1, :],
                    mybir.AluOpType.mult,
                    mybir.AluOpType.add,
                )
            stv = st_ts[gb][:, ti, :]
            nc.vector.tensor_tensor(stv, w_sb[:], onem[:], mybir.AluOpType.is_gt)
            if t < T - 1:  # the final carry update is dead code
                p_t = scr.tile([128, O_S8], f32, tag="p")
                nc.vector.scalar_tensor_tensor(
                    p_t[:],
                    w_sb[:],
                    BETA,
                    bcur[:],
                    mybir.AluOpType.mult,
                    mybir.AluOpType.add,
                )
                nc.vector.tensor_tensor(w_sb[:], p_t[:], stv, mybir.AluOpType.subtract)

            # spikes leave in half-batches (quarters at the very end):
            # keeps the out-queue smooth and shortens the final drain tail.
            if ti == TQ // 2 - 1:
                nc.sync.dma_start(
                    out=spk_r[gb, :, : TQ // 2], in_=st_ts[gb][:, : TQ // 2, :]
                )
            elif gb == NB - 1 and ti == 5:
                nc.sync.dma_start(out=spk_r[gb, :, 4:6], in_=st_ts[gb][:, 4:6, :])
            elif ti == TQ - 1:
                lo = 6 if gb == NB - 1 else TQ // 2
                nc.sync.dma_start(
                    out=spk_r[gb, :, lo:], in_=st_ts[gb][:, lo:, :]
                )

    _slim_waits(nc)
    _split_waits(nc)
    return nc


def _prepare_in_maps_dr8(x, W, b):
    import ml_dtypes

    f8 = ml_dtypes.float8_e4m3
    x = np.ascontiguousarray(x, dtype=np.float32)
    W = np.ascontiguousarray(W, dtype=np.float32)
    b = np.ascontiguousarray(b, dtype=np.float32)

    x2 = x.reshape(T * B, NI)
    xT = np.ascontiguousarray(x2.T)
    xh = xT.astype(np.float16)
    xl8 = ((xT - xh.astype(np.float32)) * 2.0**16).astype(f8)

    in_maps = []
    for c in range(NCORES):
        Wc = W[c * O_S8 : (c + 1) * O_S8, :]
        WT = np.ascontiguousarray(Wc.T)  # [NI, O_S8]
        Wh = WT.astype(np.float16)
        Wl = WT - Wh.astype(np.float32)
        w8l = (Wl * 2.0**21).astype(f8)  # pairs with fp8(xh) (plane 0)
        bc = b[c * O_S8 : (c + 1) * O_S8]
        bt = np.empty((2, 128, O_S8), np.float32)
        bt[0] = 1.0 - bc
        bt[1] = BETA * bc
        in_maps.append({"xh": xh, "xl8": xl8, "wh": Wh, "w8l": w8l, "bt": bt})
    return in_maps


def _build_nc_dr():
    from contextlib import ExitStack

    import concourse.bass as bass
    import concourse.mybir as mybir
    import concourse.tile as tile

    f32 = mybir.dt.float32
    f16 = mybir.dt.float16
    f8 = mybir.dt.float8e4
    DR = mybir.MatmulPerfMode.DoubleRow
    TB = T * B_S

    nc = bass.Bass()
    xh = nc.declare_dram_parameter("xh", [NI, TB], f16, isOutput=False)
    x8 = nc.declare_dram_parameter("x8", [2, NI, TB], f8, isOutput=False)
    wh = nc.declare_dram_parameter("wh", [NI, O_S], f16, isOutput=False)
    w8 = nc.declare_dram_parameter("w8", [2, NI, O_S], f8, isOutput=False)
    # plane 0: 1 - b (bcast over partitions); plane 1: beta * b
    bt = nc.declare_dram_parameter("bt", [2, 128, O_S], f32, isOutput=False)
    spk = nc.declare_dram_parameter("spk", [T, B_S, O_S], f16, isOutput=True)

    with tile.TileContext(nc) as tc, ExitStack() as ctx:
        singles = ctx.enter_context(tc.tile_pool(name="singles", bufs=1))
        xhp = ctx.enter_context(tc.tile_pool(name="xhp", bufs=2))
        x8p = ctx.enter_context(tc.tile_pool(name="x8p", bufs=2))
        stp = ctx.enter_context(tc.tile_pool(name="stp", bufs=2))
        scr = ctx.enter_context(tc.tile_pool(name="scr", bufs=2))
        pmp = ctx.enter_context(tc.tile_pool(name="pmp", bufs=3, space="PSUM"))
        pcp = ctx.enter_context(tc.tile_pool(name="pcp", bufs=4, space="PSUM"))

        xhr = xh[:].rearrange("(k p) tb -> p k tb", p=128)
        x8r = x8[:].rearrange("h (k p) tb -> p h k tb", p=128)

        # DMA issue order sets arrival order on the queue: the fp16 operands
        # (xh batch 0, Wh) first so pass-1 matmuls start ~11us in, then the
        # fp8 operands for the DoubleRow groups.
        xh_t0 = xhp.tile([128, KC, G * 128], f16)
        nc.sync.dma_start(out=xh_t0[:], in_=xhr[:, :, : G * 128])
        wh_sb = singles.tile([128, KC, O_S], f16)
        nc.sync.dma_start(out=wh_sb[:], in_=wh[:].rearrange("(k p) o -> p k o", p=128))
        x8_t0 = x8p.tile([128, 2, KC, G * 128], f8)
        nc.sync.dma_start(out=x8_t0[:], in_=x8r[:, :, :, : G * 128])
        w8_sb = singles.tile([128, 2, KC, O_S], f8)
        nc.sync.dma_start(
            out=w8_sb[:, 0], in_=w8l[:].rearrange("(k p) o -> p k o", p=128)
        )
        bias_sb = singles.tile([128, 2, O_S], f32)
        nc.sync.dma_start(out=bias_sb[:], in_=bt[:].rearrange("h p o -> p h o"))

        w_sb = singles.tile([64, O_S], f32)  # carry: beta*m - spk, per (b, o)
        nc.vector.memset(w_sb[:], 0.0)

        spk_r = spk[:].rearrange("(gb gi s) b o -> gb (s b) gi o", gi=G, s=2)

        def emit_f(xh_t, gi):
            ps_m = pmp.tile([128, O_S], f32, tag="m")
            cw = slice(gi * 128, (gi + 1) * 128)
            for k in range(KC):
                nc.tensor.matmul(
                    ps_m[:],
                    lhsT=xh_t[:, k, cw],
                    rhs=wh_sb[:, k, :],
                    start=(k == 0),
                    stop=(k == KC - 1),
                )
            return ps_m

        def emit_d(x8_t, gi):
            ps_c = pcp.tile([128, O_S], f32, tag="c")
            cw = slice(gi * 128, (gi + 1) * 128)
            for k in range(KC):
                nc.tensor.matmul(
                    ps_c[:],
                    lhsT=x8_t[:, :, k, cw],
                    rhs=w8_sb[:, :, k, :],
                    start=(k == 0),
                    stop=(k == KC - 1),
                    perf_mode=DR,
                )
            return ps_c

        def emit_feeds(ps_m, ps_c):
            # curb = ps_m + 2^-CSH*ps_c + b, then the bias-folded scan
            # tensors; stt ops cannot take two PSUM sources, so the otherwise
            # idle Act engine downscales the correction PSUM.
            c1 = scr.tile([128, O_S], f32, tag="c1", bufs=3)
            nc.scalar.activation(
                c1[:], ps_c[:], mybir.ActivationFunctionType.Copy, scale=2.0**-CSH
            )
            c0 = scr.tile([128, O_S], f32, tag="c0", bufs=3)
            nc.vector.tensor_tensor(c0[:], c1[:], ps_m[:], mybir.AluOpType.add)
            onem = scr.tile([128, O_S], f32, tag="onem", bufs=3)
            nc.gpsimd.scalar_tensor_tensor(
                onem[:],
                c0[:],
                -1.0,
                bias_sb[:, 0, :],
                mybir.AluOpType.mult,
                mybir.AluOpType.add,
            )
            bcur = scr.tile([128, O_S], f32, tag="bcur", bufs=3)
            nc.gpsimd.scalar_tensor_tensor(
                bcur[:],
                c0[:],
                BETA,
                bias_sb[:, 1, :],
                mybir.AluOpType.mult,
                mybir.AluOpType.add,
            )
            return onem, bcur

        def emit_state(onem, bcur, st_t, gi):
            for s in range(2):
                ph = slice(s * 64, (s + 1) * 64)
                stv = st_t[ph, gi, :]
                nc.vector.tensor_tensor(stv, w_sb[:], onem[ph, :], mybir.AluOpType.is_gt)
                p_t = scr.tile([64, O_S], f32, tag="p", bufs=4)
                nc.gpsimd.scalar_tensor_tensor(
                    p_t[:],
                    w_sb[:],
                    BETA,
                    bcur[ph, :],
                    mybir.AluOpType.mult,
                    mybir.AluOpType.add,
                )
                nc.vector.tensor_tensor(w_sb[:], p_t[:], stv, mybir.AluOpType.subtract)

        # The scan is software-pipelined one M-group deep: group g's feed ops
        # (PSUM combine + bias folds, no serial state dependency) are emitted
        # BEFORE group g-1's state-update ops so the engine FIFOs never force
        # the feeds behind the w-chain. The true critical cycle is then just
        # st/p -> w per sub-step (~3.4us), under the PE's 5.1us per group.
        PIPE = 2  # scan pipeline depth in M-groups
        pending = []  # deferred (onem, bcur, st_t, gi, gb) awaiting state ops
        xh_t = x8_t = st_t = None
        ps_ms = {}
        for g in range(NG):
            gb, gi = divmod(g, G)
            if gi == 0:
                if gb == 0:
                    xh_t, x8_t = xh_t0, x8_t0
                else:
                    xh_t = xhp.tile([128, KC, G * 128], f16)
                    nc.sync.dma_start(
                        out=xh_t[:], in_=xhr[:, :, gb * G * 128 : (gb + 1) * G * 128]
                    )
                    x8_t = x8p.tile([128, 2, KC, G * 128], f8)
                    nc.sync.dma_start(
                        out=x8_t[:],
                        in_=x8r[:, :, :, gb * G * 128 : (gb + 1) * G * 128],
                    )
                st_t = stp.tile([128, G, O_S], f16)
            if gb == 0:
                # Batch 0: run fp16 groups up to 3 ahead of the DoubleRow
                # groups so the PE (in-order) isn't idled by the fp8 operand
                # preload, which queues behind the fp16 one on the DMA queue.
                # The 3-ahead fp16 group is emitted after this group's DR
                # matmuls: its PSUM slot frees only once this group's PSUM
                # combine has run, which itself needs the DR result.
                if g == 0:
                    for ahead in range(3):
                        ps_ms[ahead] = emit_f(xh_t, ahead)
                ps_m = ps_ms.pop(g)
                ps_c = emit_d(x8_t, gi)
                if g + 3 < G:
                    ps_ms[g + 3] = emit_f(xh_t, g + 3)
            else:
                ps_m = emit_f(xh_t, gi)
                ps_c = emit_d(x8_t, gi)
            onem, bcur = emit_feeds(ps_m, ps_c)
            pending.append((onem, bcur, st_t, gi, gb))
            if len(pending) > PIPE:
                po, pb, pst, pgi, pgb = pending.pop(0)
                emit_state(po, pb, pst, pgi)
                if pgi == G - 1:  # finished writing batch pgb's st tile
                    nc.sync.dma_start(out=spk_r[pgb], in_=pst[:])
        for po, pb, pst, pgi, pgb in pending:
            emit_state(po, pb, pst, pgi)
            if pgi == G - 1:
                nc.sync.dma_start(out=spk_r[pgb], in_=pst[:])

    _slim_waits(nc)
    _split_waits(nc)
    return nc


def _build_nc_fp16x2():
    """Previous-generation kernel: pure output sharding, fp16x2 3-pass GEMM.

    Kept for A/B timing. O_S8 = 256 outputs per core, x replicated.
    """
    from contextlib import ExitStack

    import concourse.bass as bass
    import concourse.mybir as mybir
    import concourse.tile as tile

    f32 = mybir.dt.float32
    dt_mm = mybir.dt.float16
    O_S8 = NO // NCORES
    KC8 = NI // 128

    nc = bass.Bass()
    n_planes = 2
    xT = nc.declare_dram_parameter("xT", [n_planes, NI, T * B], dt_mm, isOutput=False)
    WTs = nc.declare_dram_parameter("WTs", [n_planes, NI, O_S8], dt_mm, isOutput=False)
    ob = nc.declare_dram_parameter(
        "ob", [1, 128 + n_planes * O_S8], dt_mm, isOutput=False
    )
    spk = nc.declare_dram_parameter("spk", [T, B, O_S8], f32, isOutput=True)

    TQ = 4
    with tile.TileContext(nc) as tc, ExitStack() as ctx:
        singles = ctx.enter_context(tc.tile_pool(name="singles", bufs=1))
        xpool = ctx.enter_context(tc.tile_pool(name="xp", bufs=2))
        spool = ctx.enter_context(tc.tile_pool(name="sp", bufs=3))
        sbpool = ctx.enter_context(tc.tile_pool(name="sb", bufs=2))
        psum = ctx.enter_context(tc.tile_pool(name="ps", bufs=6, space="PSUM"))

        xTr = xT[:].rearrange("h (k p) tb -> p h k tb", p=128)
        xt0 = xpool.tile([128, n_planes, KC8, TQ * B], dt_mm)
        nc.sync.dma_start(out=xt0[:], in_=xTr[:, :, :, : TQ * B])
        wt_sb = singles.tile([128, n_planes, KC8, O_S8], dt_mm)
        WTr = WTs[:].rearrange("h (k p) o -> p h k o", p=128)
        for h in range(n_planes):
            nc.sync.dma_start(out=wt_sb[:, h], in_=WTr[:, h])
        ob_sb = singles.tile([1, 128 + n_planes * O_S8], dt_mm)
        nc.sync.dma_start(out=ob_sb[:], in_=ob[:])

        m_sb = singles.tile([128, O_S8], f32)
        w_sb = singles.tile([128, O_S8], f32)
        bias_full = singles.tile([128, O_S8], f32)
        ps_b = psum.tile([128, O_S8], f32, tag="c")
        for h in range(n_planes):
            nc.tensor.matmul(
                ps_b[:],
                lhsT=ob_sb[:, :128],
                rhs=ob_sb[:, 128 + h * O_S8 : 128 + (h + 1) * O_S8],
                start=(h == 0),
                stop=(h == n_planes - 1),
            )
        nc.vector.tensor_copy(bias_full[:], ps_b[:])
        nc.vector.tensor_copy(w_sb[:], bias_full[:])

        spk_r = spk[:].rearrange("(tq tt) b o -> tq b tt o", tt=TQ)

        for tq in range(T // TQ):
            if tq == 0:
                xt = xt0
            else:
                xt = xpool.tile([128, n_planes, KC8, TQ * B], dt_mm)
                nc.sync.dma_start(
                    out=xt[:], in_=xTr[:, :, :, tq * TQ * B : (tq + 1) * TQ * B]
                )
            st = spool.tile([128, TQ, O_S8], f32)

            for tt in range(TQ):
                ps = psum.tile([128, O_S8], f32, tag="c")
                passes = ((0, 0), (0, 1), (1, 0))
                mms = [(k, hx, hw) for k in range(KC8) for hx, hw in passes]
                for i, (k, hx, hw) in enumerate(mms):
                    nc.tensor.matmul(
                        ps[:],
                        lhsT=xt[:, hx, k, tt * B : (tt + 1) * B],
                        rhs=wt_sb[:, hw, k, :],
                        start=(i == 0),
                        stop=(i == len(mms) - 1),
                    )
                nc.vector.tensor_tensor(m_sb[:], w_sb[:], ps[:], mybir.AluOpType.add)
                nc.vector.tensor_scalar(
                    st[:, tt, :], m_sb[:], 1.0, None, mybir.AluOpType.is_gt
                )
                sb = sbpool.tile([128, O_S8], f32)
                nc.vector.tensor_tensor(
                    sb[:], st[:, tt, :], bias_full[:], mybir.AluOpType.subtract
                )
                nc.vector.scalar_tensor_tensor(
                    w_sb[:],
                    m_sb[:],
                    BETA,
                    sb[:],
                    mybir.AluOpType.mult,
                    mybir.AluOpType.subtract,
                )
            nc.sync.dma_start(out=spk_r[tq], in_=st[:])

    _slim_waits(nc)
    _split_waits(nc)
    return nc


def _build_nc(mode):
    if mode == "dr8":
        return _build_nc_dr8()
    if mode == "dr":
        return _build_nc_dr()
    return _build_nc_fp16x2()


def _slim_waits(nc):
    """Drop sync waits already implied by earlier ones (transitive closure).

    Each engine queue dispatches in FIFO order, so a wait satisfied on an
    earlier instruction of the same queue covers later instructions. A wait
    on sem s >= v also imports everything the incrementing instruction's
    queue had itself waited for when it raised s to v (semaphore vector
    clocks with snapshots at each increment).
    """
    FRAMEWORK_OPS = ("InstEventSemaphore", "InstDrain")
    engine_clock = {}  # engine -> {sem_id: value known reached}
    totals = {}  # sem_id -> running total of increments
    snapshots = {}  # sem_id -> [(value, clock dict)] in increasing value order
    poisoned = set()  # sems touched by non-monotonic updates (barriers)

    def join(dst, src):
        for s, v in src.items():
            if s in poisoned:
                continue
            if dst.get(s, -1) < v:
                dst[s] = v

    for blk in nc.m.functions[0].blocks:
        for inst in blk.instructions:
            si = getattr(inst, "sync_info", None)
            if si is None:
                continue
            is_framework = type(inst).__name__ in FRAMEWORK_OPS
            clock = engine_clock.setdefault(inst.engine, {})
            if si.on_wait:
                kept = []
                for w in si.on_wait:
                    if (
                        w.sync_type != "semaphore"
                        or w.wait_mode != "sem-ge-imm"
                        or w.id in poisoned
                    ):
                        kept.append(w)
                        continue
                    covered = clock.get(w.id, -1) >= w.wait_value
                    for val, snap in snapshots.get(w.id, ()):
                        if val <= w.wait_value:
                            join(clock, snap)
                        else:
                            break
                    if clock.get(w.id, -1) < w.wait_value:
                        clock[w.id] = w.wait_value
                    if is_framework or not covered:
                        kept.append(w)
                si.on_wait = kept
            if si.on_update:
                for u in si.on_update:
                    if u.sync_type != "semaphore":
                        continue
                    if u.update_mode not in ("sem-inc", "sem-add-imm"):
                        # barrier-style sem: stop reasoning about it entirely
                        poisoned.add(u.id)
                        totals.pop(u.id, None)
                        snapshots.pop(u.id, None)
                        for c in engine_clock.values():
                            c.pop(u.id, None)
                        continue
                    if u.id in poisoned:
                        continue
                    tot = totals.get(u.id, 0) + (u.update_value or 1)
                    totals[u.id] = tot
                    snap = dict(clock)
                    snap[u.id] = tot
                    snapshots.setdefault(u.id, []).append((tot, snap))


def _split_waits(nc, limit=1):
    """Move excess sync waits onto injected same-queue NoOps.

    Walrus codegen accepts at most `limit` sync-wait commands per engine
    instruction on this target. Engine queues dispatch in order, so a
    preceding NoOp carrying the wait is equivalent.
    """
    import concourse.mybir as mybir

    n_nops = 0
    for blk in nc.m.functions[0].blocks:
        out = []
        changed = False
        for inst in blk.instructions:
            si = getattr(inst, "sync_info", None)
            if type(inst).__name__ == "InstEventSemaphore":
                out.append(inst)
                continue
            if si is not None and si.on_wait and len(si.on_wait) > limit:
                waits = list(si.on_wait)
                for w in waits[:-limit]:
                    nop = mybir.InstNoOp(name=f"wnop-{n_nops}", ins=[], outs=[])
                    n_nops += 1
                    nop.engine = inst.engine
                    nop.sync_info = mybir.SyncInfo(on_wait=[w], on_update=[])
                    nop.bass_nofuse = True
                    out.append(nop)
                    changed = True
                si.on_wait = waits[-limit:]
            out.append(inst)
        if changed:
            try:
                blk.instructions = out
            except Exception:
                blk.instructions.clear()
                blk.instructions.extend(out)


def _split16(a):
    hi = a.astype(np.float16)
    lo = (a - hi.astype(np.float32)).astype(np.float16)
    return hi, lo


def _prepare_in_maps_dr(x, W, b):
    import ml_dtypes

    f8 = ml_dtypes.float8_e4m3
    x = np.ascontiguousarray(x, dtype=np.float32)
    W = np.ascontiguousarray(W, dtype=np.float32)
    b = np.ascontiguousarray(b, dtype=np.float32)

    # per batch group: xh [NI, T*B_S] fp16, x8 [2, NI, T*B_S] fp8
    xh_bg, x8_bg = [], []
    for bg in range(BGN):
        xc = x[:, bg * B_S : (bg + 1) * B_S, :].reshape(T * B_S, NI)
        xT = np.ascontiguousarray(xc.T)
        xh = xT.astype(np.float16)
        xl = xT - xh.astype(np.float32)
        x8 = np.empty((2, NI, T * B_S), f8)
        x8[0] = (xh.astype(np.float32) * 2.0**4).astype(f8)
        x8[1] = (xl * 2.0**16).astype(f8)
        xh_bg.append(xh)
        x8_bg.append(x8)

    # per output group: wh [NI, O_S] fp16, w8 [2, NI, O_S] fp8, bias tiles
    wh_og, w8_og, bt_og = [], [], []
    for og in range(OGN):
        Wc = W[og * O_S : (og + 1) * O_S, :]
        WT = np.ascontiguousarray(Wc.T)  # [NI, O_S]
        Wh = WT.astype(np.float16)
        Wl = WT - Wh.astype(np.float32)
        w8 = np.empty((2, NI, O_S), f8)
        w8[0] = (Wl * 2.0**19).astype(f8)  # pairs with xh8 (plane 0)
        w8[1] = (Wh.astype(np.float32) * 2.0**7).astype(f8)  # pairs with xl8
        bc = b[og * O_S : (og + 1) * O_S]
        bt = np.empty((2, 128, O_S), np.float32)
        bt[0] = 1.0 - bc
        bt[1] = BETA * bc
        wh_og.append(Wh)
        w8_og.append(w8)
        bt_og.append(bt)

    in_maps = []
    for c in range(NCORES):
        bg, og = divmod(c, OGN)
        in_maps.append(
            {
                "xh": xh_bg[bg],
                "x8": x8_bg[bg],
                "wh": wh_og[og],
                "w8": w8_og[og],
                "bt": bt_og[og],
            }
        )
    return in_maps


def _prepare_in_maps_fp16x2(x, W, b):
    O_S8 = NO // NCORES
    x = np.ascontiguousarray(x, dtype=np.float32)
    W = np.ascontiguousarray(W, dtype=np.float32)
    b = np.ascontiguousarray(b, dtype=np.float32)
    x2 = x.reshape(T * B, NI)
    xh, xl = _split16(x2)
    xT = np.stack([np.ascontiguousarray(xh.T), np.ascontiguousarray(xl.T)])
    Wh, Wl = _split16(W)
    WTs_full = np.stack([np.ascontiguousarray(Wh.T), np.ascontiguousarray(Wl.T)])
    bh, bl = _split16(b)
    b_planes = [bh, bl]
    n_planes = 2
    in_maps = []
    for c in range(NCORES):
        ob = np.empty((1, 128 + n_planes * O_S8), np.float16)
        ob[0, :128] = 1.0
        for h in range(n_planes):
            ob[0, 128 + h * O_S8 : 128 + (h + 1) * O_S8] = b_planes[h][
                c * O_S8 : (c + 1) * O_S8
            ]
        in_maps.append(
            {
                "xT": xT,
                "WTs": np.ascontiguousarray(WTs_full[:, :, c * O_S8 : (c + 1) * O_S8]),
                "ob": ob,
            }
        )
    return in_maps


def run(x, W, b, trace=False):
    """Run the kernel; returns (out [T,B,NO] fp32, BassKernelResults)."""
    from concourse.bass_utils import run_bass_kernel_spmd

    if MODE not in _cache:
        _cache[MODE] = _build_nc(MODE)
    nc = _cache[MODE]
    if MODE == "dr8":
        in_maps = _prepare_in_maps_dr8(x, W, b)
    elif MODE == "dr":
        in_maps = _prepare_in_maps_dr(x, W, b)
    else:
        in_maps = _prepare_in_maps_fp16x2(x, W, b)
    res = run_bass_kernel_spmd(nc, in_maps, list(range(NCORES)), trace=trace)
    if MODE == "dr8":
        out = np.empty((T, B, NO), np.float32)
        for c in range(NCORES):
            out[:, :, c * O_S8 : (c + 1) * O_S8] = res.results[c]["spk"].astype(
                np.float32
            )
    elif MODE == "dr":
        out = np.empty((T, B, NO), np.float32)
        for c in range(NCORES):
            bg, og = divmod(c, OGN)
            out[:, bg * B_S : (bg + 1) * B_S, og * O_S : (og + 1) * O_S] = res.results[
                c
            ]["spk"].astype(np.float32)
    else:
        out = np.concatenate([res.results[c]["spk"] for c in range(NCORES)], axis=2)
    return out, res


def kernel(x, W, b):
    out, _ = run(x, W, b, trace=False)
    return out
